# revision 113
# baseline (speedup 1.0000x reference)
"""DGCNN-style edge-conv block (KNN graph + dense conv stack) on 8 trn2 cores.

Strategy (data-parallel over batch, one batch element per core):
  scores   = 2<xi,xj> - ||xj||^2 via one fp16 PE matmul with [2x; -1] x [x; x^2]
             contraction (the -||xi||^2 term is a per-row constant and cannot
             change a row's top-k, so it is dropped).
  top-16   = int32 bit-packing: ACT evacuates q = int32(psum*512 + 2^18)
             (positive 19-bit), DVE packs (q << 12) | j. Non-negative int32
             bit patterns order identically under an fp32 view, so max8 /
             match_replace on the bitcast yield values AND indices
             (j = packed & 4095). Top-16 = 8x max8 over disjoint 256-wide
             eighths + exact 64-wide merge (max8 / match_replace / max8);
             only rows with >8 of their true top-16 in one eighth (~1e-4 of
             rows) can lose a tail neighbor.
  gather   = P^T table (P = W1a @ x, 64 ch fp16 = 128B rows) in DRAM,
             gathered per 8192-edge super-chunk with gpsimd dma_gather
             (mlp ucode library, single_packet=False).
  edge MLP = A = relu(P_j + T_n), B2 = relu(W2a A + R_n),
             C3 = W3a A + W3c B2 + S_n, with T/R/S = per-node tables from
             small fp16 matmuls; per-edge convs run as fp16 block-diag
             matmuls on PE with 2k-stacked PE transposes. The R bias is
             folded into the conv2 PSUM group as an identity x R-broadcast
             fp16 matmul. C3 is never evacuated: its k-max first level reads
             the two conv3 PSUM halves directly.
  output   = channel-concat [max_k A; x; max_k B2; max_k C3] in fp16
             (host upconverts to fp32); k-max trees run as fp16
             tensor_tensor trees (2x DVE mode) with the cross-half merge
             fused into one scalar_tensor_tensor (4x mode).

Schedule: all 16 row-tiles' scores+topk are emitted first; each super-chunk's
gather/transpose/conv/max stages trail behind on DMA, PE and ACT as soon as
its 4 index tiles are ready.
"""

import numpy as np

import bass_rust
import concourse.bass as bass
import concourse.bass_isa as bass_isa
import concourse.mybir as mybir
from concourse.bass_types import AP
from concourse.tile import TileContext
from concourse.bass_utils import run_bass_kernel_spmd

F32 = mybir.dt.float32
F16 = mybir.dt.float16
I32 = mybir.dt.int32
U16 = mybir.dt.uint16
I16 = mybir.dt.int16

B, C, N, K, G = 8, 64, 2048, 16, 64
NT = 16          # 128-row tiles
NSC = 4          # super-chunks
NBL = 4          # nblocks per super-chunk
RELU = mybir.ActivationFunctionType.Relu
COPY = mybir.ActivationFunctionType.Copy
SQUARE = mybir.ActivationFunctionType.Square
ADD = mybir.AluOpType.add
MAX = mybir.AluOpType.max

_nop_ctr = [0]


def _split_all_waits(nc, max_waits=1):
    # This walrus build rejects >1 sync-wait on several CTRL structs; hoist
    # extras onto single-wait NOPs placed just before the instruction.
    for fn in nc.m.functions:
        for bb in fn.blocks:
            out = []
            for ins in bb.instructions:
                si = ins.sync_info
                if si is not None and si.on_wait is not None and len(si.on_wait) > max_waits:
                    waits = list(si.on_wait)
                    for w in waits[:-max_waits]:
                        _nop_ctr[0] += 1
                        nop = mybir.InstNoOp(name=f"waitnop-{_nop_ctr[0]}", ins=[], outs=[])
                        nop.engine = ins.engine
                        nop.sync_info = bass_rust.SyncInfo(on_wait=[w], on_update=[])
                        out.append(nop)
                        nc.register_instruction(nop, overwrite=True)
                    si.on_wait = waits[-max_waits:]
                out.append(ins)
            bb.instructions = out


def _fix_int_imms(nc):
    # walrus requires bitvec-op immediates to be integer-typed and match the
    # src/dst dtype; bass lowers python ints to float32 ImmVals, so retype
    # the immediates on int32 shift/bitwise TensorScalarPtr ops.
    bitvec = (mybir.AluOpType.logical_shift_left,
              mybir.AluOpType.logical_shift_right,
              mybir.AluOpType.arith_shift_left,
              mybir.AluOpType.arith_shift_right,
              mybir.AluOpType.bitwise_and,
              mybir.AluOpType.bitwise_or,
              mybir.AluOpType.bitwise_xor)
    for fn in nc.m.functions:
        for bb in fn.blocks:
            for ins in bb.instructions:
                if not isinstance(ins, mybir.InstTensorScalarPtr):
                    continue
                if ins.op0 not in bitvec and getattr(ins, "op1", None) not in bitvec:
                    continue
                new_ins = list(ins.ins)
                changed = False
                for i, a in enumerate(new_ins):
                    if isinstance(a, mybir.ImmediateValue) and a.dtype != mybir.dt.int32:
                        new_ins[i] = mybir.ImmediateValue(dtype=mybir.dt.int32,
                                                          value=int(a.value))
                        changed = True
                if changed:
                    ins.ins = new_ins


def _insert_gpsimd_library_load(nc, lib_index=3):
    # InstDMAGatherAnt needs the 'mlp' GPSIMD ucode library; raw Bass+Tile
    # skips Bacc's insert_library_loads, so prepend the reload by hand.
    ins = bass_isa.InstPseudoReloadLibraryIndex(
        name="libload-manual", ins=[], outs=[], lib_index=lib_index
    )
    ins.engine = mybir.EngineType.Pool
    nc.register_instruction(ins, overwrite=True)
    bb0 = nc.m.functions[0].blocks[0]
    bb0.instructions = [ins] + list(bb0.instructions)
    mybir.codegen_inst_isa_subclasses(nc)


def build():
    nc = bass.Bass("TRN2", debug=False, num_devices=8)

    x_in = nc.dram_tensor("x", [C, N], F32, kind="ExternalInput")
    IOTAI = nc.dram_tensor("IOTAI", [128, N], I32, kind="ExternalInput")
    WLTP = nc.dram_tensor("WLTP", [64, 64], F16, kind="ExternalInput")    # W1a.T
    WLT = nc.dram_tensor("WLT", [65, 64], F16, kind="ExternalInput")      # [(W1b-W1a).T; b1]
    WLR = nc.dram_tensor("WLR", [65, 64], F16, kind="ExternalInput")      # [W2b.T; b2]
    WLS = nc.dram_tensor("WLS", [65, 64], F16, kind="ExternalInput")      # [W3b.T; b3]
    W2BLK = nc.dram_tensor("W2BLK", [128, 128], F16, kind="ExternalInput")
    W3ABLK = nc.dram_tensor("W3ABLK", [128, 128], F16, kind="ExternalInput")
    W3CBLK = nc.dram_tensor("W3CBLK", [128, 128], F16, kind="ExternalInput")
    EYE = nc.dram_tensor("EYE16", [128, 128], F16, kind="ExternalInput")
    EYE32 = nc.dram_tensor("EYE32", [128, 128], F32, kind="ExternalInput")
    Y = nc.dram_tensor("y", [C + 3 * G, N], F16, kind="ExternalOutput")

    PT_D = nc.dram_tensor("PT_D", [N, 64], F32, kind="Internal")
    IDXD = nc.dram_tensor("IDXD", [N * K], I16, kind="Internal")

    with TileContext(nc) as tc:
        with tc.tile_pool(name="const", bufs=1) as cp, \
             tc.tile_pool(name="work", bufs=2) as wp, \
             tc.tile_pool(name="chunk", bufs=1) as kp, \
             tc.tile_pool(name="gat", bufs=2) as gp, \
             tc.tile_pool(name="psS", bufs=2, space="PSUM") as ppsS, \
             tc.tile_pool(name="psA", bufs=1, space="PSUM") as ppsA, \
             tc.tile_pool(name="psU", bufs=1, space="PSUM") as ppsU, \
             tc.tile_pool(name="psC", bufs=2, space="PSUM") as ppsC:

            # ---------------- setup ----------------
            X65 = cp.tile([65, N], F32)
            X16 = cp.tile([65, N], F16)
            RHSB = cp.tile([128, N], F16)
            LHSB = cp.tile([128, N], F16)
            IOTAt = cp.tile([128, N], I32)
            PC = cp.tile([64, N], F32)
            TSTK = cp.tile([128, N], F16)
            RSTK = cp.tile([128, N], F16)
            SCt = cp.tile([64, N], F16)
            PTS = cp.tile([128, NT * 64], F32)
            IDXALL = cp.tile([128, NT * K], U16)
            EYE16 = cp.tile([128, 128], F16)
            EYE32t = cp.tile([128, 128], F32)
            wltp = cp.tile([64, 64], F16)
            wlt = cp.tile([65, 64], F16)
            wlr = cp.tile([65, 64], F16)
            wls = cp.tile([65, 64], F16)
            w2b = cp.tile([128, 128], F16)
            w3a = cp.tile([128, 128], F16)
            w3c = cp.tile([128, 128], F16)

            nc.sync.dma_start(out=X65[0:64, 0:1024], in_=x_in[:, 0:1024])
            nc.sync.dma_start(out=IOTAt[:, 0:1024], in_=IOTAI[:, 0:1024])
            nc.sync.dma_start(out=X65[0:64, 1024:2048], in_=x_in[:, 1024:2048])
            nc.sync.dma_start(out=IOTAt[:, 1024:2048], in_=IOTAI[:, 1024:2048])
            nc.sync.dma_start(out=EYE16[:, :], in_=EYE[:, :])
            nc.sync.dma_start(out=EYE32t[:, :], in_=EYE32[:, :])
            nc.sync.dma_start(out=wltp[:, :], in_=WLTP[:, :])
            nc.sync.dma_start(out=wlt[:, :], in_=WLT[:, :])
            nc.sync.dma_start(out=wlr[:, :], in_=WLR[:, :])
            nc.sync.dma_start(out=wls[:, :], in_=WLS[:, :])
            nc.sync.dma_start(out=w2b[:, :], in_=W2BLK[:, :])
            nc.sync.dma_start(out=w3a[:, :], in_=W3ABLK[:, :])
            nc.sync.dma_start(out=w3c[:, :], in_=W3CBLK[:, :])
            nc.gpsimd.memset(X16[64:65, :], 1.0)
            nc.gpsimd.memset(LHSB[64:128, :], -1.0)

            # Startup conversions run on the (otherwise idle) DVE in halves so
            # tile 0's score matmuls start as soon as each x half lands.
            MUL = mybir.AluOpType.mult
            for half in range(2):
                hs = slice(half * 1024, (half + 1) * 1024)
                nc.vector.tensor_scalar(out=RHSB[0:64, hs], in0=X65[0:64, hs],
                                        scalar1=1.0, scalar2=None, op0=MUL)
                nc.vector.tensor_tensor(out=RHSB[64:128, hs], in0=X65[0:64, hs],
                                        in1=X65[0:64, hs], op=MUL)
                nc.vector.tensor_scalar(out=LHSB[0:64, hs], in0=X65[0:64, hs],
                                        scalar1=2.0, scalar2=None, op0=MUL)
                nc.vector.tensor_scalar(out=X16[0:64, hs], in0=X65[0:64, hs],
                                        scalar1=1.0, scalar2=None, op0=MUL)

            def emit_ptab(u):
                # P (c-layout) chunk u + its 4 P^T-table tiles; spread across
                # the first topk tile group so the ACT/PE work hides behind
                # the DVE-bound topk stream.
                sl = slice(u * 512, (u + 1) * 512)
                p1 = ppsU.tile([64, 512], F32, tag="u2")
                nc.tensor.matmul(p1[:, :], wltp[:, :], X16[0:64, sl], start=True, stop=True)
                nc.scalar.activation(PC[:, sl], p1[:, :], COPY)
                for rt in range(4 * u, 4 * u + 4):
                    pt = ppsA.tile([128, 512], F32, tag="a")
                    nc.tensor.transpose(pt[:, 0:64], PC[:, rt * 128:(rt + 1) * 128],
                                        EYE32t[0:64, 0:64])
                    nc.scalar.activation(PTS[:, rt * 64:(rt + 1) * 64], pt[:, 0:64], COPY)
                if u == 3:
                    nc.sync.dma_start(
                        out=AP(PT_D, 0, [[64, 128], [8192, NT], [1, 64]]),
                        in_=PTS[:, :].rearrange("p (a b) -> p a b", a=NT),
                    )
                    # x passthrough output rows 64:128 (fp16)
                    nc.sync.dma_start(out=Y[64:128, :], in_=X16[0:64, :])

            def emit_trs(u):
                # T/R stacked and S table chunk u; spread across the second
                # topk tile group (only needed by the first super-chunk).
                sl = slice(u * 512, (u + 1) * 512)
                p2 = ppsU.tile([64, 512], F32, tag="u2")
                nc.tensor.matmul(p2[:, :], wlt[:, :], X16[:, sl], start=True, stop=True)
                nc.scalar.activation(TSTK[0:64, sl], p2[:, :], COPY)
                p3 = ppsU.tile([64, 512], F32, tag="u2")
                nc.tensor.matmul(p3[:, :], wlr[:, :], X16[:, sl], start=True, stop=True)
                nc.scalar.activation(RSTK[0:64, sl], p3[:, :], COPY)
                p4 = ppsU.tile([64, 512], F32, tag="u2")
                nc.tensor.matmul(p4[:, :], wls[:, :], X16[:, sl], start=True, stop=True)
                nc.scalar.activation(SCt[:, sl], p4[:, :], COPY)
                nc.scalar.activation(TSTK[64:128, sl], TSTK[0:64, sl], COPY)
                nc.scalar.activation(RSTK[64:128, sl], RSTK[0:64, sl], COPY)

            # ---------------- scores + topk for one row tile ----------------
            # See module docstring: int32 (score<<12 | j) packing, fp32-view
            # max8 over eighths + exact merge, j = packed & 4095.
            def emit_topk(rt):
                if True:
                    PACKED = wp.tile([128, N], I32, tag="scores")
                    for u in range(4):
                        pss = ppsS.tile([128, 512], F32, tag="score")
                        nc.tensor.matmul(pss[:, :],
                                         LHSB[:, rt * 128:(rt + 1) * 128],
                                         RHSB[:, u * 512:(u + 1) * 512],
                                         start=True, stop=True)
                        nc.scalar.activation(PACKED[:, u * 512:(u + 1) * 512],
                                             pss[:, :], COPY,
                                             scale=512.0, bias=262144.0)
                    CAND = wp.tile([128, 64], F32, tag="cand")
                    T16 = wp.tile([128, 16], F32, tag="t16")
                    for half in range(2):
                        hs = slice(half * 1024, (half + 1) * 1024)
                        nc.vector.scalar_tensor_tensor(
                            out=PACKED[:, hs], in0=PACKED[:, hs], scalar=12,
                            in1=IOTAt[:, hs],
                            op0=mybir.AluOpType.logical_shift_left,
                            op1=mybir.AluOpType.bitwise_or)
                        for e in range(4 * half, 4 * half + 4):
                            nc.vector.max(out=CAND[:, 8 * e:8 * e + 8],
                                          in_=PACKED[:, 256 * e:256 * (e + 1)].bitcast(F32))
                    nc.vector.max(out=T16[:, 0:8], in_=CAND[:, :])
                    nc.vector.match_replace(out=CAND[:, :], in_to_replace=T16[:, 0:8],
                                            in_values=CAND[:, :], imm_value=0.0)
                    nc.vector.max(out=T16[:, 8:16], in_=CAND[:, :])
                    # bitvec ops need src/dst dtypes equal, so AND into i32
                    # then convert to u16 with an arithmetic op.
                    T16I = wp.tile([128, 16], I32, tag="t16i")
                    nc.vector.tensor_scalar(out=T16I[:, :],
                                            in0=T16[:, :].bitcast(I32),
                                            scalar1=4095, scalar2=None,
                                            op0=mybir.AluOpType.bitwise_and)
                    nc.vector.tensor_scalar(out=IDXALL[:, rt * K:(rt + 1) * K],
                                            in0=T16I[:, :],
                                            scalar1=0, scalar2=None,
                                            op0=mybir.AluOpType.add)

            # ---------------- per node-tile gather + edge MLP block ----------
            # One gather per 128-node tile (2048 edges): the idx chain starts
            # right after the tile's own topk extract and the 4x-smaller
            # transfer pipelines tile-by-tile instead of serializing 12us
            # slabs on the DMA track.
            RED = {}
            BLK = {}

            PGD = {}

            def emit_gather(pr):
                # One gather per PAIR of node tiles (4096 edges): amortizes
                # the idx write/read/replication hop latency over two tiles.
                # IDXD layout: addr = pr*4096 + r*32 + t*16 + k (t = tile
                # parity; contiguous 64B runs per partition on the write).
                nc.sync.dma_start(
                    out=AP(IDXD, pr * 4096, [[32, 128], [16, 2], [1, K]]),
                    in_=IDXALL[:, 2 * pr * K:(2 * pr + 2) * K].bitcast(I16)
                        .rearrange("p (t k) -> p t k", t=2),
                )
                idxt = gp.tile([128, 256], I16, tag="idxt")
                # idxt[g*16+m, s'], s' = j'*8 + c, j' = t*16 + j:
                #   <- addr pr*4096 + (m+16c)*32 + t*16 + j
                src_w = AP(IDXD, pr * 4096,
                           [[32, 16], [16, 2], [1, K], [512, 8]])
                nc.sync.dma_start(out=idxt[0:16, :], in_=src_w)
                for lo, hi in ((16, 32), (32, 64), (64, 128)):
                    nc.sync.dma_start(out=idxt[lo:hi, :], in_=idxt[0:lo, :])

                pgnew = gp.tile([128, 2 * K, 64], F32, tag="pg")
                PGD[2 * pr] = (pgnew, 0)
                PGD[2 * pr + 1] = (pgnew, K)
                nc.gpsimd.dma_gather(
                    out_ap=pgnew[:, :, :], in_ap=PT_D.ap(), idxs_ap=idxt[:, :],
                    num_idxs=4096, num_idxs_reg=4096, elem_size=64,
                    single_packet=False,
                )

            def emit_gather_single(rt):
                # Single-tile gather (2048 edges) for the LAST two tiles: the
                # tail cannot hide a pair chain behind later topk work, so
                # tile 14's block starts as soon as its own indices exist.
                # IDXD layout: addr = rt*2048 + r*16 + k.
                nc.sync.dma_start(
                    out=AP(IDXD, rt * 2048, [[16, 128], [1, K]]),
                    in_=IDXALL[:, rt * K:(rt + 1) * K].bitcast(I16),
                )
                idxt = gp.tile([128, 256], I16, tag="idxt")
                # idxt[g*16+m, s'], s' = j*8 + c <- addr rt*2048 + (m+16c)*16 + j
                src_w = AP(IDXD, rt * 2048, [[16, 16], [1, K], [256, 8]])
                nc.sync.dma_start(out=idxt[0:16, 0:128], in_=src_w)
                for lo, hi in ((16, 32), (32, 64), (64, 128)):
                    nc.sync.dma_start(out=idxt[lo:hi, 0:128], in_=idxt[0:lo, 0:128])

                pgnew = gp.tile([128, 2 * K, 64], F32, tag="pg")
                PGD[rt] = (pgnew, 0)
                nc.gpsimd.dma_gather(
                    out_ap=pgnew[:, 0:K, :], in_ap=PT_D.ap(),
                    idxs_ap=idxt[:, 0:128],
                    num_idxs=2048, num_idxs_reg=2048, elem_size=64,
                    single_packet=False,
                )

            def emit_block(rt):
                sc, bl = rt // 4, rt % 4
                pgt, joff = PGD.pop(rt)
                PG = pgt[:, joff:joff + K, :]

                AC = kp.tile([128, 8, 128], F16, tag=f"ac{rt % 2}")
                B2C = kp.tile([128, 8, 128], F16, tag=f"b2c{rt % 2}")
                tb = TSTK[:, rt * 128:(rt + 1) * 128].unsqueeze(1).broadcast_to([128, 4, 128])
                rb = RSTK[:, rt * 128:(rt + 1) * 128].unsqueeze(1).broadcast_to([128, 4, 128])
                for q in range(2):
                    # transposes: 4 kp blocks -> psum (128, 512)
                    psa = ppsA.tile([128, 512], F32, tag="a")
                    for kk in range(4):
                        kpi = q * 4 + kk
                        blk = PG[:, 2 * kpi:2 * kpi + 2, :]
                        nc.tensor.transpose(psa[:, kk * 128:(kk + 1) * 128],
                                            blk, EYE32t[:, :])
                    sa = wp.tile([128, 512], F16, tag="sa")
                    nc.vector.scalar_tensor_tensor(out=sa[:, :], in0=psa[:, :],
                                                   scalar=0.0, in1=tb,
                                                   op0=ADD, op1=ADD)
                    nc.scalar.activation(AC[:, 4 * q:4 * q + 4, :], sa[:, :], RELU)

                    # conv2 (+R folded in as an identity-matmul accumulate)
                    ps2t = ppsU.tile([128, 512], F32, tag="u2")
                    nc.tensor.matmul(ps2t[:, :], w2b[:, :],
                                     AC[:, 4 * q:4 * q + 4, :],
                                     start=True, stop=False,
                                     skip_group_check=True)
                    nc.tensor.matmul(ps2t[:, :].rearrange("p (a b) -> p a b", a=4),
                                     EYE16[:, :], rb,
                                     start=False, stop=True,
                                     skip_group_check=True)
                    nc.scalar.activation(B2C[:, 4 * q:4 * q + 4, :], ps2t[:, :], RELU)

                # conv3 for both halves into one psum tile; first k-max level
                # (k pairs 4 apart) reads the psum halves directly.
                psc = ppsC.tile([128, 1024], F32, tag="c3")
                for q in range(2):
                    nc.tensor.matmul(psc[:, q * 512:(q + 1) * 512], w3a[:, :],
                                     AC[:, 4 * q:4 * q + 4, :],
                                     start=True, stop=False)
                    nc.tensor.matmul(psc[:, q * 512:(q + 1) * 512], w3c[:, :],
                                     B2C[:, 4 * q:4 * q + 4, :],
                                     start=False, stop=True)
                # DVE may read only one PSUM operand: evacuate the q1 half so
                # the C3 first-level max pairs psum against SBUF.
                c3h = kp.tile([128, 4, 128], F16, tag=f"c3h{rt % 2}")
                nc.scalar.activation(c3h[:, :, :],
                                     psc[:, 512:1024].rearrange("p (a b) -> p a b", a=4),
                                     COPY)
                BLK[rt] = (AC, B2C, psc, c3h)

            def emit_trees(rt):
                # k-max trees (fp16, 2x DVE), deferred one tile so every
                # input (relu evacs, conv3 psum) is long done when the DVE
                # stream reaches them — no cross-engine ping-pong stalls.
                sc, bl = rt // 4, rt % 4
                AC, B2C, psc, c3h = BLK.pop(rt)
                for (src, row0, lv) in ((AC, 0, 3), (B2C, 2 * G, 3), (psc, 3 * G, 2)):
                    if bl == 0:
                        rednew = kp.tile([128, NBL, 128], F16,
                                         tag=f"red{row0}{sc % 2}")
                        RED[(sc, row0)] = rednew
                    red = RED[(sc, row0)]
                    if lv == 3:
                        m1 = kp.tile([128, 4, 128], F16, tag=f"m1{row0}")
                        nc.vector.tensor_tensor(out=m1[:, :, :], in0=src[:, 0:4, :],
                                                in1=src[:, 4:8, :], op=MAX)
                    else:
                        m1 = kp.tile([128, 4, 128], F16, tag=f"m1{row0}")
                        nc.vector.tensor_tensor(
                            out=m1[:, :, :],
                            in0=src[:, 0:512].rearrange("p (a b) -> p a b", a=4),
                            in1=c3h[:, :, :],
                            op=MAX)
                    m2 = kp.tile([128, 2, 128], F16, tag=f"m2{row0}")
                    nc.vector.tensor_tensor(out=m2[:, :, :], in0=m1[:, 0:2, :],
                                            in1=m1[:, 2:4, :], op=MAX)
                    nc.vector.tensor_tensor(out=red[:, bl, :], in0=m2[:, 0, :],
                                            in1=m2[:, 1, :], op=MAX)

            def emit_om(sc):
                # cross-half merge (4x stt) + S bias + output DMAs, per sc.
                for (row0, add_s) in ((0, False), (2 * G, False), (3 * G, True)):
                    red = RED[(sc, row0)]
                    hi = kp.tile([64, NBL * 128], F16, tag=f"hi{row0}")
                    nc.scalar.activation(hi[:, :],
                                         red[64:128, :, :].rearrange("p a n -> p (a n)"),
                                         COPY)
                    om = kp.tile([64, NBL * 128], F16, tag=f"om{row0}")
                    nc.vector.tensor_tensor(
                        out=om[:, :],
                        in0=red[0:64, :, :].rearrange("p a n -> p (a n)"),
                        in1=hi[:, :], op=MAX)
                    if add_s:
                        om2 = kp.tile([64, NBL * 128], F16, tag="oms")
                        nc.vector.tensor_tensor(out=om2[:, :], in0=om[:, :],
                                                in1=SCt[:, sc * 512:(sc + 1) * 512],
                                                op=ADD)
                        om = om2
                    nc.sync.dma_start(out=Y[row0 if row0 else 0:(row0 if row0 else 0) + 64,
                                            sc * 512:(sc + 1) * 512],
                                      in_=om[:, :])

            # Interleaved emission: engines execute their streams in emission
            # order, so super-chunk work is placed one tile-group behind the
            # topk tiles whose indices it needs — sc_i's gather round-trip
            # hides behind tile group i+1's topk, and the table setup spreads
            # across groups 0 (P table) and 1 (T/R/S tables).
            for rt in range(NT):
                emit_topk(rt)
                # P table over tiles 0-2 (PT_D written before the first
                # gather's idx DMAs in queue order); T/R/S chunk u at tile 4u
                # (chunk u is first needed by block 4u at iteration 4u+3).
                if rt == 0:
                    emit_ptab(0)
                elif rt == 1:
                    emit_ptab(1)
                    emit_ptab(2)
                elif rt == 2:
                    emit_ptab(3)
                # pair gathers; pr=0 is deferred one tile so the PT_D table
                # write (ptab(3), tile 2) precedes it in DMA-queue order.
                if rt == 2:
                    emit_gather(0)
                elif rt % 2 == 1 and 3 <= rt <= 13:
                    emit_gather(rt // 2)
                elif rt >= 14:
                    emit_gather_single(rt)
                if rt >= 4:
                    emit_block(rt - 4)
                if rt >= 5:
                    emit_trees(rt - 5)
                    if (rt - 5) % 4 == 3:
                        emit_om((rt - 5) // 4)
                # T/R/S after block work: the PE reaches the DVE-blocking
                # transposes before burning time on the tables.
                if rt % 4 == 0:
                    emit_trs(rt // 4)
            for br in (NT - 4, NT - 3, NT - 2, NT - 1):
                emit_block(br)
                emit_trees(br - 1)
                if (br - 1) % 4 == 3:
                    emit_om((br - 1) // 4)
            emit_trees(NT - 1)
            emit_om(3)

    _fix_int_imms(nc)
    _split_all_waits(nc)
    _insert_gpsimd_library_load(nc, 3)
    return nc


def _prep_weights(W1, b1, W2, b2, W3, b3):
    W1 = np.asarray(W1, np.float32); W2 = np.asarray(W2, np.float32)
    W3 = np.asarray(W3, np.float32)
    b1 = np.asarray(b1, np.float32); b2 = np.asarray(b2, np.float32)
    b3 = np.asarray(b3, np.float32)
    W1a, W1b = W1[:, :64], W1[:, 64:]
    W2a, W2b = W2[:, :64], W2[:, 64:]
    W3a, W3b, W3c = W3[:, :64], W3[:, 64:128], W3[:, 128:]

    def blk(w):
        z = np.zeros((128, 128), np.float16)
        z[0:64, 0:64] = w.T
        z[64:128, 64:128] = w.T
        return z

    f16 = np.float16
    return {
        "WLTP": np.ascontiguousarray(W1a.T).astype(f16),
        "WLT": np.ascontiguousarray(np.vstack([(W1b - W1a).T, b1[None, :]])).astype(f16),
        "WLR": np.ascontiguousarray(np.vstack([W2b.T, b2[None, :]])).astype(f16),
        "WLS": np.ascontiguousarray(np.vstack([W3b.T, b3[None, :]])).astype(f16),
        "W2BLK": blk(W2a),
        "W3ABLK": blk(W3a),
        "W3CBLK": blk(W3c),
        "EYE16": np.eye(128, dtype=f16),
        "EYE32": np.eye(128, dtype=np.float32),
        "IOTAI": np.tile(np.arange(N, dtype=np.int32), (128, 1)),
    }


_NC = None


def kernel(x, W1, b1, W2, b2, W3, b3):
    global _NC
    if _NC is None:
        _NC = build()
    x = np.asarray(x, np.float32)
    w = _prep_weights(W1, b1, W2, b2, W3, b3)
    in_maps = [{"x": np.ascontiguousarray(x[b]), **w} for b in range(B)]
    res = run_bass_kernel_spmd(_NC, in_maps, core_ids=list(range(B)))
    return np.stack([res.results[b]["y"].astype(np.float32) for b in range(B)], axis=0)


# revision 116
# speedup vs baseline: 1.0513x; 1.0513x over previous
"""DGCNN-style edge-conv block (KNN graph + dense conv stack) on 8 trn2 cores.

Strategy (data-parallel over batch, one batch element per core):
  scores   = 2<xi,xj> - ||xj||^2 via one fp16 PE matmul with [2x; -1] x [x; x^2]
             contraction (the -||xi||^2 term is a per-row constant and cannot
             change a row's top-k, so it is dropped).
  top-16   = int32 bit-packing: ACT evacuates q = int32(psum*512 + 2^18)
             (positive 19-bit), DVE packs (q << 12) | j. Non-negative int32
             bit patterns order identically under an fp32 view, so max8 /
             match_replace on the bitcast yield values AND indices
             (j = packed & 4095). Top-16 = 8x max8 over disjoint 256-wide
             eighths + exact 64-wide merge (max8 / match_replace / max8);
             only rows with >8 of their true top-16 in one eighth (~1e-4 of
             rows) can lose a tail neighbor.
  gather   = P^T table (P = W1a @ x, 64 ch fp16 = 128B rows) in DRAM,
             gathered per 8192-edge super-chunk with gpsimd dma_gather
             (mlp ucode library, single_packet=False).
  edge MLP = A = relu(P_j + T_n), B2 = relu(W2a A + R_n),
             C3 = W3a A + W3c B2 + S_n, with T/R/S = per-node tables from
             small fp16 matmuls; per-edge convs run as fp16 block-diag
             matmuls on PE with 2k-stacked PE transposes. The R bias is
             folded into the conv2 PSUM group as an identity x R-broadcast
             fp16 matmul. C3 is never evacuated: its k-max first level reads
             the two conv3 PSUM halves directly.
  output   = channel-concat [max_k A; x; max_k B2; max_k C3] in fp16
             (host upconverts to fp32); k-max trees run as fp16
             tensor_tensor trees (2x DVE mode) with the cross-half merge
             fused into one scalar_tensor_tensor (4x mode).

Schedule: all 16 row-tiles' scores+topk are emitted first; each super-chunk's
gather/transpose/conv/max stages trail behind on DMA, PE and ACT as soon as
its 4 index tiles are ready.
"""

import numpy as np

import bass_rust
import concourse.bass as bass
import concourse.bass_isa as bass_isa
import concourse.mybir as mybir
from concourse.bass_types import AP
from concourse.tile import TileContext
from concourse.bass_utils import run_bass_kernel_spmd

F32 = mybir.dt.float32
F16 = mybir.dt.float16
I32 = mybir.dt.int32
U16 = mybir.dt.uint16
I16 = mybir.dt.int16

B, C, N, K, G = 8, 64, 2048, 16, 64
NT = 16          # 128-row tiles
NSC = 4          # super-chunks
NBL = 4          # nblocks per super-chunk
RELU = mybir.ActivationFunctionType.Relu
COPY = mybir.ActivationFunctionType.Copy
SQUARE = mybir.ActivationFunctionType.Square
ADD = mybir.AluOpType.add
MAX = mybir.AluOpType.max

_nop_ctr = [0]


def _split_all_waits(nc, max_waits=1):
    # This walrus build rejects >1 sync-wait on several CTRL structs; hoist
    # extras onto single-wait NOPs placed just before the instruction.
    for fn in nc.m.functions:
        for bb in fn.blocks:
            out = []
            for ins in bb.instructions:
                si = ins.sync_info
                if si is not None and si.on_wait is not None and len(si.on_wait) > max_waits:
                    waits = list(si.on_wait)
                    for w in waits[:-max_waits]:
                        _nop_ctr[0] += 1
                        nop = mybir.InstNoOp(name=f"waitnop-{_nop_ctr[0]}", ins=[], outs=[])
                        nop.engine = ins.engine
                        nop.sync_info = bass_rust.SyncInfo(on_wait=[w], on_update=[])
                        out.append(nop)
                        nc.register_instruction(nop, overwrite=True)
                    si.on_wait = waits[-max_waits:]
                out.append(ins)
            bb.instructions = out


def _fix_int_imms(nc):
    # walrus requires bitvec-op immediates to be integer-typed and match the
    # src/dst dtype; bass lowers python ints to float32 ImmVals, so retype
    # the immediates on int32 shift/bitwise TensorScalarPtr ops.
    bitvec = (mybir.AluOpType.logical_shift_left,
              mybir.AluOpType.logical_shift_right,
              mybir.AluOpType.arith_shift_left,
              mybir.AluOpType.arith_shift_right,
              mybir.AluOpType.bitwise_and,
              mybir.AluOpType.bitwise_or,
              mybir.AluOpType.bitwise_xor)
    for fn in nc.m.functions:
        for bb in fn.blocks:
            for ins in bb.instructions:
                if not isinstance(ins, mybir.InstTensorScalarPtr):
                    continue
                if ins.op0 not in bitvec and getattr(ins, "op1", None) not in bitvec:
                    continue
                new_ins = list(ins.ins)
                changed = False
                for i, a in enumerate(new_ins):
                    if isinstance(a, mybir.ImmediateValue) and a.dtype != mybir.dt.int32:
                        new_ins[i] = mybir.ImmediateValue(dtype=mybir.dt.int32,
                                                          value=int(a.value))
                        changed = True
                if changed:
                    ins.ins = new_ins


def _insert_gpsimd_library_load(nc, lib_index=3):
    # InstDMAGatherAnt needs the 'mlp' GPSIMD ucode library; raw Bass+Tile
    # skips Bacc's insert_library_loads, so prepend the reload by hand.
    ins = bass_isa.InstPseudoReloadLibraryIndex(
        name="libload-manual", ins=[], outs=[], lib_index=lib_index
    )
    ins.engine = mybir.EngineType.Pool
    nc.register_instruction(ins, overwrite=True)
    bb0 = nc.m.functions[0].blocks[0]
    bb0.instructions = [ins] + list(bb0.instructions)
    mybir.codegen_inst_isa_subclasses(nc)


def build():
    nc = bass.Bass("TRN2", debug=False, num_devices=8)

    x_in = nc.dram_tensor("x", [C, N], F32, kind="ExternalInput")
    IOTAI = nc.dram_tensor("IOTAI", [128, N], I32, kind="ExternalInput")
    WLTP = nc.dram_tensor("WLTP", [64, 64], F16, kind="ExternalInput")    # W1a.T
    WLT = nc.dram_tensor("WLT", [65, 64], F16, kind="ExternalInput")      # [(W1b-W1a).T; b1]
    WLTD = nc.dram_tensor("WLTD", [65, 128], F16, kind="ExternalInput")   # [WLT | WLT]
    WLR = nc.dram_tensor("WLR", [65, 64], F16, kind="ExternalInput")      # [W2b.T; b2]
    WLS = nc.dram_tensor("WLS", [65, 64], F16, kind="ExternalInput")      # [W3b.T; b3]
    W2BLK = nc.dram_tensor("W2BLK", [128, 128], F16, kind="ExternalInput")
    W3ABLK = nc.dram_tensor("W3ABLK", [128, 128], F16, kind="ExternalInput")
    W3CBLK = nc.dram_tensor("W3CBLK", [128, 128], F16, kind="ExternalInput")
    EYE = nc.dram_tensor("EYE16", [128, 128], F16, kind="ExternalInput")
    EYE32 = nc.dram_tensor("EYE32", [128, 128], F32, kind="ExternalInput")
    Y = nc.dram_tensor("y", [C + 3 * G, N], F16, kind="ExternalOutput")

    PT_D = nc.dram_tensor("PT_D", [N, 64], F32, kind="Internal")
    IDXD = nc.dram_tensor("IDXD", [N * K], I16, kind="Internal")

    with TileContext(nc) as tc:
        with tc.tile_pool(name="const", bufs=1) as cp, \
             tc.tile_pool(name="work", bufs=2) as wp, \
             tc.tile_pool(name="chunk", bufs=1) as kp, \
             tc.tile_pool(name="gat", bufs=2) as gp, \
             tc.tile_pool(name="psS", bufs=2, space="PSUM") as ppsS, \
             tc.tile_pool(name="psA", bufs=1, space="PSUM") as ppsA, \
             tc.tile_pool(name="psU", bufs=1, space="PSUM") as ppsU, \
             tc.tile_pool(name="psC", bufs=2, space="PSUM") as ppsC:

            # ---------------- setup ----------------
            X65 = cp.tile([65, N], F32)
            X16 = cp.tile([65, N], F16)
            RHSB = cp.tile([128, N], F16)
            LHSB = cp.tile([128, N], F16)
            IOTAt = cp.tile([128, N], I32)
            PC = cp.tile([64, N], F32)
            TSTK = cp.tile([128, N], F16)
            RSTK = cp.tile([128, N], F16)
            SCt = cp.tile([64, N], F16)
            PTS = cp.tile([128, NT * 64], F32)
            IDXALL = cp.tile([128, NT * K], U16)
            EYE16 = cp.tile([128, 128], F16)
            EYE32t = cp.tile([128, 128], F32)
            wltp = cp.tile([64, 64], F16)
            wlt = cp.tile([65, 64], F16)
            wltd = cp.tile([65, 128], F16)
            wlr = cp.tile([65, 64], F16)
            wls = cp.tile([65, 64], F16)
            w2b = cp.tile([128, 128], F16)
            w3a = cp.tile([128, 128], F16)
            w3c = cp.tile([128, 128], F16)

            nc.sync.dma_start(out=X65[0:64, 0:1024], in_=x_in[:, 0:1024])
            nc.sync.dma_start(out=IOTAt[:, 0:1024], in_=IOTAI[:, 0:1024])
            nc.sync.dma_start(out=X65[0:64, 1024:2048], in_=x_in[:, 1024:2048])
            nc.sync.dma_start(out=IOTAt[:, 1024:2048], in_=IOTAI[:, 1024:2048])
            nc.sync.dma_start(out=EYE16[:, :], in_=EYE[:, :])
            nc.sync.dma_start(out=EYE32t[:, :], in_=EYE32[:, :])
            nc.sync.dma_start(out=wltp[:, :], in_=WLTP[:, :])
            nc.sync.dma_start(out=wlt[:, :], in_=WLT[:, :])
            nc.sync.dma_start(out=wltd[:, :], in_=WLTD[:, :])
            nc.sync.dma_start(out=wlr[:, :], in_=WLR[:, :])
            nc.sync.dma_start(out=wls[:, :], in_=WLS[:, :])
            nc.sync.dma_start(out=w2b[:, :], in_=W2BLK[:, :])
            nc.sync.dma_start(out=w3a[:, :], in_=W3ABLK[:, :])
            nc.sync.dma_start(out=w3c[:, :], in_=W3CBLK[:, :])
            nc.gpsimd.memset(X16[64:65, :], 1.0)
            nc.gpsimd.memset(LHSB[64:128, :], -1.0)

            # Startup conversions run on the (otherwise idle) DVE in halves so
            # tile 0's score matmuls start as soon as each x half lands.
            MUL = mybir.AluOpType.mult
            for half in range(2):
                hs = slice(half * 1024, (half + 1) * 1024)
                nc.vector.tensor_scalar(out=RHSB[0:64, hs], in0=X65[0:64, hs],
                                        scalar1=1.0, scalar2=None, op0=MUL)
                nc.vector.tensor_tensor(out=RHSB[64:128, hs], in0=X65[0:64, hs],
                                        in1=X65[0:64, hs], op=MUL)
                nc.vector.tensor_scalar(out=LHSB[0:64, hs], in0=X65[0:64, hs],
                                        scalar1=2.0, scalar2=None, op0=MUL)
                nc.vector.tensor_scalar(out=X16[0:64, hs], in0=X65[0:64, hs],
                                        scalar1=1.0, scalar2=None, op0=MUL)

            def emit_ptab(u):
                # P (c-layout) chunk u + its 4 P^T-table tiles; spread across
                # the first topk tile group so the ACT/PE work hides behind
                # the DVE-bound topk stream.
                sl = slice(u * 512, (u + 1) * 512)
                p1 = ppsU.tile([64, 512], F32, tag="u2")
                nc.tensor.matmul(p1[:, :], wltp[:, :], X16[0:64, sl], start=True, stop=True)
                nc.scalar.activation(PC[:, sl], p1[:, :], COPY)
                for rt in range(4 * u, 4 * u + 4):
                    pt = ppsA.tile([128, 512], F32, tag="a")
                    nc.tensor.transpose(pt[:, 0:64], PC[:, rt * 128:(rt + 1) * 128],
                                        EYE32t[0:64, 0:64])
                    nc.scalar.activation(PTS[:, rt * 64:(rt + 1) * 64], pt[:, 0:64], COPY)
                if u == 3:
                    nc.sync.dma_start(
                        out=AP(PT_D, 0, [[64, 128], [8192, NT], [1, 64]]),
                        in_=PTS[:, :].rearrange("p (a b) -> p a b", a=NT),
                    )
                    # x passthrough output rows 64:128 (fp16)
                    nc.sync.dma_start(out=Y[64:128, :], in_=X16[0:64, :])

            def emit_trs(u):
                # T/R stacked and S table chunk u; spread across the second
                # topk tile group (only needed by the first super-chunk).
                sl = slice(u * 512, (u + 1) * 512)
                p3 = ppsU.tile([64, 512], F32, tag="u2")
                nc.tensor.matmul(p3[:, :], wlr[:, :], X16[:, sl], start=True, stop=True)
                nc.scalar.activation(RSTK[0:64, sl], p3[:, :], COPY)
                p4 = ppsU.tile([64, 512], F32, tag="u2")
                nc.tensor.matmul(p4[:, :], wls[:, :], X16[:, sl], start=True, stop=True)
                nc.scalar.activation(SCt[:, sl], p4[:, :], COPY)
                nc.scalar.activation(RSTK[64:128, sl], RSTK[0:64, sl], COPY)

            # ---------------- scores + topk for one row tile ----------------
            # See module docstring: int32 (score<<12 | j) packing, fp32-view
            # max8 over eighths + exact merge, j = packed & 4095.
            def emit_topk(rt):
                if True:
                    PACKED = wp.tile([128, N], I32, tag="scores")
                    for u in range(4):
                        pss = ppsS.tile([128, 512], F32, tag="score")
                        nc.tensor.matmul(pss[:, :],
                                         LHSB[:, rt * 128:(rt + 1) * 128],
                                         RHSB[:, u * 512:(u + 1) * 512],
                                         start=True, stop=True)
                        nc.scalar.activation(PACKED[:, u * 512:(u + 1) * 512],
                                             pss[:, :], COPY,
                                             scale=512.0, bias=262144.0)
                    CAND = wp.tile([128, 64], F32, tag="cand")
                    T16 = wp.tile([128, 16], F32, tag="t16")
                    for half in range(2):
                        hs = slice(half * 1024, (half + 1) * 1024)
                        nc.vector.scalar_tensor_tensor(
                            out=PACKED[:, hs], in0=PACKED[:, hs], scalar=12,
                            in1=IOTAt[:, hs],
                            op0=mybir.AluOpType.logical_shift_left,
                            op1=mybir.AluOpType.bitwise_or)
                        for e in range(4 * half, 4 * half + 4):
                            nc.vector.max(out=CAND[:, 8 * e:8 * e + 8],
                                          in_=PACKED[:, 256 * e:256 * (e + 1)].bitcast(F32))
                    nc.vector.max(out=T16[:, 0:8], in_=CAND[:, :])
                    nc.vector.match_replace(out=CAND[:, :], in_to_replace=T16[:, 0:8],
                                            in_values=CAND[:, :], imm_value=0.0)
                    nc.vector.max(out=T16[:, 8:16], in_=CAND[:, :])
                    # bitvec ops need src/dst dtypes equal, so AND into i32
                    # then convert to u16 with an arithmetic op.
                    T16I = wp.tile([128, 16], I32, tag="t16i")
                    nc.vector.tensor_scalar(out=T16I[:, :],
                                            in0=T16[:, :].bitcast(I32),
                                            scalar1=4095, scalar2=None,
                                            op0=mybir.AluOpType.bitwise_and)
                    nc.vector.tensor_scalar(out=IDXALL[:, rt * K:(rt + 1) * K],
                                            in0=T16I[:, :],
                                            scalar1=0, scalar2=None,
                                            op0=mybir.AluOpType.add)

            # ---------------- per node-tile gather + edge MLP block ----------
            # One gather per 128-node tile (2048 edges): the idx chain starts
            # right after the tile's own topk extract and the 4x-smaller
            # transfer pipelines tile-by-tile instead of serializing 12us
            # slabs on the DMA track.
            RED = {}
            BLK = {}

            PGD = {}

            def emit_gather(pr):
                # One gather per PAIR of node tiles (4096 edges): amortizes
                # the idx write/read/replication hop latency over two tiles.
                # IDXD layout: addr = pr*4096 + r*32 + t*16 + k (t = tile
                # parity; contiguous 64B runs per partition on the write).
                nc.sync.dma_start(
                    out=AP(IDXD, pr * 4096, [[32, 128], [16, 2], [1, K]]),
                    in_=IDXALL[:, 2 * pr * K:(2 * pr + 2) * K].bitcast(I16)
                        .rearrange("p (t k) -> p t k", t=2),
                )
                idxt = gp.tile([128, 256], I16, tag="idxt")
                # idxt[g*16+m, s'], s' = j'*8 + c, j' = t*16 + j:
                #   <- addr pr*4096 + (m+16c)*32 + t*16 + j
                src_w = AP(IDXD, pr * 4096,
                           [[32, 16], [16, 2], [1, K], [512, 8]])
                nc.sync.dma_start(out=idxt[0:16, :], in_=src_w)
                for lo, hi in ((16, 32), (32, 64), (64, 128)):
                    nc.sync.dma_start(out=idxt[lo:hi, :], in_=idxt[0:lo, :])

                pgnew = gp.tile([128, 2 * K, 64], F32, tag="pg")
                PGD[2 * pr] = (pgnew, 0)
                PGD[2 * pr + 1] = (pgnew, K)
                nc.gpsimd.dma_gather(
                    out_ap=pgnew[:, :, :], in_ap=PT_D.ap(), idxs_ap=idxt[:, :],
                    num_idxs=4096, num_idxs_reg=4096, elem_size=64,
                    single_packet=False,
                )

            def emit_gather_single(rt):
                # Single-tile gather (2048 edges) for the LAST two tiles: the
                # tail cannot hide a pair chain behind later topk work, so
                # tile 14's block starts as soon as its own indices exist.
                # IDXD layout: addr = rt*2048 + r*16 + k.
                nc.sync.dma_start(
                    out=AP(IDXD, rt * 2048, [[16, 128], [1, K]]),
                    in_=IDXALL[:, rt * K:(rt + 1) * K].bitcast(I16),
                )
                idxt = gp.tile([128, 256], I16, tag="idxt")
                # idxt[g*16+m, s'], s' = j*8 + c <- addr rt*2048 + (m+16c)*16 + j
                src_w = AP(IDXD, rt * 2048, [[16, 16], [1, K], [256, 8]])
                nc.sync.dma_start(out=idxt[0:16, 0:128], in_=src_w)
                for lo, hi in ((16, 32), (32, 64), (64, 128)):
                    nc.sync.dma_start(out=idxt[lo:hi, 0:128], in_=idxt[0:lo, 0:128])

                pgnew = gp.tile([128, 2 * K, 64], F32, tag="pg")
                PGD[rt] = (pgnew, 0)
                nc.gpsimd.dma_gather(
                    out_ap=pgnew[:, 0:K, :], in_ap=PT_D.ap(),
                    idxs_ap=idxt[:, 0:128],
                    num_idxs=2048, num_idxs_reg=2048, elem_size=64,
                    single_packet=False,
                )

            def emit_block(rt):
                sc, bl = rt // 4, rt % 4
                pgt, joff = PGD.pop(rt)
                PG = pgt[:, joff:joff + K, :]

                AC = kp.tile([128, 8, 128], F16, tag=f"ac{rt % 2}")
                B2C = kp.tile([128, 8, 128], F16, tag=f"b2c{rt % 2}")
                rb = RSTK[:, rt * 128:(rt + 1) * 128].unsqueeze(1).broadcast_to([128, 4, 128])
                # T^T for this tile's nodes, computed directly (one matmul:
                # X16-block^T x [wlt|wlt]); accumulated into each psa block
                # as a second is_transpose matmul so the T bias lands in the
                # conv1 PSUM and the 19us DVE add disappears — ACT's relu
                # reads the psum directly.
                ttp_ps = ppsU.tile([128, 128], F32, tag="u2")
                nc.tensor.matmul(ttp_ps[:, :], X16[:, rt * 128:(rt + 1) * 128],
                                 wltd[:, :], start=True, stop=True)
                ttp = kp.tile([128, 128], F32, tag=f"ttp{rt % 2}")
                nc.scalar.activation(ttp[:, :], ttp_ps[:, :], COPY)
                for q in range(2):
                    # transposes: 4 kp blocks + T^T -> psum (128, 512)
                    psa = ppsA.tile([128, 512], F32, tag="a")
                    for kk in range(4):
                        kpi = q * 4 + kk
                        blk = PG[:, 2 * kpi:2 * kpi + 2, :]
                        nc.tensor.matmul(psa[:, kk * 128:(kk + 1) * 128],
                                         blk, EYE32t[:, :], is_transpose=True,
                                         start=True, stop=False,
                                         skip_group_check=True)
                        nc.tensor.matmul(psa[:, kk * 128:(kk + 1) * 128],
                                         ttp[:, :], EYE32t[:, :], is_transpose=True,
                                         start=False, stop=True,
                                         skip_group_check=True)
                    nc.scalar.activation(AC[:, 4 * q:4 * q + 4, :],
                                         psa[:, :].rearrange("p (a b) -> p a b", a=4),
                                         RELU)

                    # conv2 (+R folded in as an identity-matmul accumulate)
                    ps2t = ppsU.tile([128, 512], F32, tag="u2")
                    nc.tensor.matmul(ps2t[:, :], w2b[:, :],
                                     AC[:, 4 * q:4 * q + 4, :],
                                     start=True, stop=False,
                                     skip_group_check=True)
                    nc.tensor.matmul(ps2t[:, :].rearrange("p (a b) -> p a b", a=4),
                                     EYE16[:, :], rb,
                                     start=False, stop=True,
                                     skip_group_check=True)
                    nc.scalar.activation(B2C[:, 4 * q:4 * q + 4, :], ps2t[:, :], RELU)

                # conv3 for both halves into one psum tile; first k-max level
                # (k pairs 4 apart) reads the psum halves directly.
                psc = ppsC.tile([128, 1024], F32, tag="c3")
                for q in range(2):
                    nc.tensor.matmul(psc[:, q * 512:(q + 1) * 512], w3a[:, :],
                                     AC[:, 4 * q:4 * q + 4, :],
                                     start=True, stop=False)
                    nc.tensor.matmul(psc[:, q * 512:(q + 1) * 512], w3c[:, :],
                                     B2C[:, 4 * q:4 * q + 4, :],
                                     start=False, stop=True)
                # DVE may read only one PSUM operand: evacuate the q1 half so
                # the C3 first-level max pairs psum against SBUF.
                c3h = kp.tile([128, 4, 128], F16, tag=f"c3h{rt % 2}")
                nc.scalar.activation(c3h[:, :, :],
                                     psc[:, 512:1024].rearrange("p (a b) -> p a b", a=4),
                                     COPY)
                BLK[rt] = (AC, B2C, psc, c3h)

            def emit_trees(rt):
                # k-max trees (fp16, 2x DVE), deferred one tile so every
                # input (relu evacs, conv3 psum) is long done when the DVE
                # stream reaches them — no cross-engine ping-pong stalls.
                sc, bl = rt // 4, rt % 4
                AC, B2C, psc, c3h = BLK.pop(rt)
                for (src, row0, lv) in ((AC, 0, 3), (B2C, 2 * G, 3), (psc, 3 * G, 2)):
                    if bl == 0:
                        rednew = kp.tile([128, NBL, 128], F16,
                                         tag=f"red{row0}{sc % 2}")
                        RED[(sc, row0)] = rednew
                    red = RED[(sc, row0)]
                    if lv == 3:
                        m1 = kp.tile([128, 4, 128], F16, tag=f"m1{row0}")
                        nc.vector.tensor_tensor(out=m1[:, :, :], in0=src[:, 0:4, :],
                                                in1=src[:, 4:8, :], op=MAX)
                    else:
                        m1 = kp.tile([128, 4, 128], F16, tag=f"m1{row0}")
                        nc.vector.tensor_tensor(
                            out=m1[:, :, :],
                            in0=src[:, 0:512].rearrange("p (a b) -> p a b", a=4),
                            in1=c3h[:, :, :],
                            op=MAX)
                    m2 = kp.tile([128, 2, 128], F16, tag=f"m2{row0}")
                    nc.vector.tensor_tensor(out=m2[:, :, :], in0=m1[:, 0:2, :],
                                            in1=m1[:, 2:4, :], op=MAX)
                    nc.vector.tensor_tensor(out=red[:, bl, :], in0=m2[:, 0, :],
                                            in1=m2[:, 1, :], op=MAX)

            def emit_om(sc):
                # cross-half merge (4x stt) + S bias + output DMAs, per sc.
                for (row0, add_s) in ((0, False), (2 * G, False), (3 * G, True)):
                    red = RED[(sc, row0)]
                    hi = kp.tile([64, NBL * 128], F16, tag=f"hi{row0}")
                    nc.scalar.activation(hi[:, :],
                                         red[64:128, :, :].rearrange("p a n -> p (a n)"),
                                         COPY)
                    om = kp.tile([64, NBL * 128], F16, tag=f"om{row0}")
                    nc.vector.tensor_tensor(
                        out=om[:, :],
                        in0=red[0:64, :, :].rearrange("p a n -> p (a n)"),
                        in1=hi[:, :], op=MAX)
                    if add_s:
                        om2 = kp.tile([64, NBL * 128], F16, tag="oms")
                        nc.vector.tensor_tensor(out=om2[:, :], in0=om[:, :],
                                                in1=SCt[:, sc * 512:(sc + 1) * 512],
                                                op=ADD)
                        om = om2
                    nc.sync.dma_start(out=Y[row0 if row0 else 0:(row0 if row0 else 0) + 64,
                                            sc * 512:(sc + 1) * 512],
                                      in_=om[:, :])

            # Interleaved emission: engines execute their streams in emission
            # order, so super-chunk work is placed one tile-group behind the
            # topk tiles whose indices it needs — sc_i's gather round-trip
            # hides behind tile group i+1's topk, and the table setup spreads
            # across groups 0 (P table) and 1 (T/R/S tables).
            for rt in range(NT):
                emit_topk(rt)
                # P table over tiles 0-2 (PT_D written before the first
                # gather's idx DMAs in queue order); T/R/S chunk u at tile 4u
                # (chunk u is first needed by block 4u at iteration 4u+3).
                if rt == 0:
                    emit_ptab(0)
                elif rt == 1:
                    emit_ptab(1)
                    emit_ptab(2)
                elif rt == 2:
                    emit_ptab(3)
                # pair gathers; pr=0 is deferred one tile so the PT_D table
                # write (ptab(3), tile 2) precedes it in DMA-queue order.
                if rt == 2:
                    emit_gather(0)
                elif rt % 2 == 1 and 3 <= rt <= 13:
                    emit_gather(rt // 2)
                elif rt >= 14:
                    emit_gather_single(rt)
                if rt >= 4:
                    emit_block(rt - 4)
                if rt >= 5:
                    emit_trees(rt - 5)
                    if (rt - 5) % 4 == 3:
                        emit_om((rt - 5) // 4)
                # T/R/S after block work: the PE reaches the DVE-blocking
                # transposes before burning time on the tables.
                if rt % 4 == 0:
                    emit_trs(rt // 4)
            for br in (NT - 4, NT - 3, NT - 2, NT - 1):
                emit_block(br)
                emit_trees(br - 1)
                if (br - 1) % 4 == 3:
                    emit_om((br - 1) // 4)
            emit_trees(NT - 1)
            emit_om(3)

    _fix_int_imms(nc)
    _split_all_waits(nc)
    _insert_gpsimd_library_load(nc, 3)
    return nc


def _prep_weights(W1, b1, W2, b2, W3, b3):
    W1 = np.asarray(W1, np.float32); W2 = np.asarray(W2, np.float32)
    W3 = np.asarray(W3, np.float32)
    b1 = np.asarray(b1, np.float32); b2 = np.asarray(b2, np.float32)
    b3 = np.asarray(b3, np.float32)
    W1a, W1b = W1[:, :64], W1[:, 64:]
    W2a, W2b = W2[:, :64], W2[:, 64:]
    W3a, W3b, W3c = W3[:, :64], W3[:, 64:128], W3[:, 128:]

    def blk(w):
        z = np.zeros((128, 128), np.float16)
        z[0:64, 0:64] = w.T
        z[64:128, 64:128] = w.T
        return z

    f16 = np.float16
    return {
        "WLTP": np.ascontiguousarray(W1a.T).astype(f16),
        "WLT": np.ascontiguousarray(np.vstack([(W1b - W1a).T, b1[None, :]])).astype(f16),
        "WLTD": np.ascontiguousarray(np.tile(np.vstack([(W1b - W1a).T, b1[None, :]]), (1, 2))).astype(f16),
        "WLR": np.ascontiguousarray(np.vstack([W2b.T, b2[None, :]])).astype(f16),
        "WLS": np.ascontiguousarray(np.vstack([W3b.T, b3[None, :]])).astype(f16),
        "W2BLK": blk(W2a),
        "W3ABLK": blk(W3a),
        "W3CBLK": blk(W3c),
        "EYE16": np.eye(128, dtype=f16),
        "EYE32": np.eye(128, dtype=np.float32),
        "IOTAI": np.tile(np.arange(N, dtype=np.int32), (128, 1)),
    }


_NC = None


def kernel(x, W1, b1, W2, b2, W3, b3):
    global _NC
    if _NC is None:
        _NC = build()
    x = np.asarray(x, np.float32)
    w = _prep_weights(W1, b1, W2, b2, W3, b3)
    in_maps = [{"x": np.ascontiguousarray(x[b]), **w} for b in range(B)]
    res = run_bass_kernel_spmd(_NC, in_maps, core_ids=list(range(B)))
    return np.stack([res.results[b]["y"].astype(np.float32) for b in range(B)], axis=0)


# revision 117
# speedup vs baseline: 1.0912x; 1.0379x over previous
"""DGCNN-style edge-conv block (KNN graph + dense conv stack) on 8 trn2 cores.

Strategy (data-parallel over batch, one batch element per core):
  scores   = 2<xi,xj> - ||xj||^2 via one fp16 PE matmul with [2x; -1] x [x; x^2]
             contraction (the -||xi||^2 term is a per-row constant and cannot
             change a row's top-k, so it is dropped).
  top-16   = int32 bit-packing: ACT evacuates q = int32(psum*512 + 2^18)
             (positive 19-bit), DVE packs (q << 12) | j. Non-negative int32
             bit patterns order identically under an fp32 view, so max8 /
             match_replace on the bitcast yield values AND indices
             (j = packed & 4095). Top-16 = 8x max8 over disjoint 256-wide
             eighths + exact 64-wide merge (max8 / match_replace / max8);
             only rows with >8 of their true top-16 in one eighth (~1e-4 of
             rows) can lose a tail neighbor.
  gather   = P^T table (P = W1a @ x, 64 ch fp16 = 128B rows) in DRAM,
             gathered per 8192-edge super-chunk with gpsimd dma_gather
             (mlp ucode library, single_packet=False).
  edge MLP = A = relu(P_j + T_n), B2 = relu(W2a A + R_n),
             C3 = W3a A + W3c B2 + S_n, with T/R/S = per-node tables from
             small fp16 matmuls; per-edge convs run as fp16 block-diag
             matmuls on PE with 2k-stacked PE transposes. The R bias is
             folded into the conv2 PSUM group as an identity x R-broadcast
             fp16 matmul. C3 is never evacuated: its k-max first level reads
             the two conv3 PSUM halves directly.
  output   = channel-concat [max_k A; x; max_k B2; max_k C3] in fp16
             (host upconverts to fp32); k-max trees run as fp16
             tensor_tensor trees (2x DVE mode) with the cross-half merge
             fused into one scalar_tensor_tensor (4x mode).

Schedule: all 16 row-tiles' scores+topk are emitted first; each super-chunk's
gather/transpose/conv/max stages trail behind on DMA, PE and ACT as soon as
its 4 index tiles are ready.
"""

import numpy as np

import bass_rust
import concourse.bass as bass
import concourse.bass_isa as bass_isa
import concourse.mybir as mybir
from concourse.bass_types import AP
from concourse.tile import TileContext
from concourse.bass_utils import run_bass_kernel_spmd

F32 = mybir.dt.float32
F16 = mybir.dt.float16
I32 = mybir.dt.int32
U16 = mybir.dt.uint16
I16 = mybir.dt.int16

B, C, N, K, G = 8, 64, 2048, 16, 64
NT = 16          # 128-row tiles
NSC = 4          # super-chunks
NBL = 4          # nblocks per super-chunk
RELU = mybir.ActivationFunctionType.Relu
COPY = mybir.ActivationFunctionType.Copy
SQUARE = mybir.ActivationFunctionType.Square
ADD = mybir.AluOpType.add
MAX = mybir.AluOpType.max

_nop_ctr = [0]


def _split_all_waits(nc, max_waits=1):
    # This walrus build rejects >1 sync-wait on several CTRL structs; hoist
    # extras onto single-wait NOPs placed just before the instruction.
    for fn in nc.m.functions:
        for bb in fn.blocks:
            out = []
            for ins in bb.instructions:
                si = ins.sync_info
                if si is not None and si.on_wait is not None and len(si.on_wait) > max_waits:
                    waits = list(si.on_wait)
                    for w in waits[:-max_waits]:
                        _nop_ctr[0] += 1
                        nop = mybir.InstNoOp(name=f"waitnop-{_nop_ctr[0]}", ins=[], outs=[])
                        nop.engine = ins.engine
                        nop.sync_info = bass_rust.SyncInfo(on_wait=[w], on_update=[])
                        out.append(nop)
                        nc.register_instruction(nop, overwrite=True)
                    si.on_wait = waits[-max_waits:]
                out.append(ins)
            bb.instructions = out


def _fix_int_imms(nc):
    # walrus requires bitvec-op immediates to be integer-typed and match the
    # src/dst dtype; bass lowers python ints to float32 ImmVals, so retype
    # the immediates on int32 shift/bitwise TensorScalarPtr ops.
    bitvec = (mybir.AluOpType.logical_shift_left,
              mybir.AluOpType.logical_shift_right,
              mybir.AluOpType.arith_shift_left,
              mybir.AluOpType.arith_shift_right,
              mybir.AluOpType.bitwise_and,
              mybir.AluOpType.bitwise_or,
              mybir.AluOpType.bitwise_xor)
    for fn in nc.m.functions:
        for bb in fn.blocks:
            for ins in bb.instructions:
                if not isinstance(ins, mybir.InstTensorScalarPtr):
                    continue
                if ins.op0 not in bitvec and getattr(ins, "op1", None) not in bitvec:
                    continue
                new_ins = list(ins.ins)
                changed = False
                for i, a in enumerate(new_ins):
                    if isinstance(a, mybir.ImmediateValue) and a.dtype != mybir.dt.int32:
                        new_ins[i] = mybir.ImmediateValue(dtype=mybir.dt.int32,
                                                          value=int(a.value))
                        changed = True
                if changed:
                    ins.ins = new_ins


def _insert_gpsimd_library_load(nc, lib_index=3):
    # InstDMAGatherAnt needs the 'mlp' GPSIMD ucode library; raw Bass+Tile
    # skips Bacc's insert_library_loads, so prepend the reload by hand.
    ins = bass_isa.InstPseudoReloadLibraryIndex(
        name="libload-manual", ins=[], outs=[], lib_index=lib_index
    )
    ins.engine = mybir.EngineType.Pool
    nc.register_instruction(ins, overwrite=True)
    bb0 = nc.m.functions[0].blocks[0]
    bb0.instructions = [ins] + list(bb0.instructions)
    mybir.codegen_inst_isa_subclasses(nc)


def build():
    nc = bass.Bass("TRN2", debug=False, num_devices=8)

    x_in = nc.dram_tensor("x", [C, N], F32, kind="ExternalInput")
    IOTAI = nc.dram_tensor("IOTAI", [128, N], I32, kind="ExternalInput")
    WLTP = nc.dram_tensor("WLTP", [64, 64], F16, kind="ExternalInput")    # W1a.T
    WLT = nc.dram_tensor("WLT", [65, 64], F16, kind="ExternalInput")      # [(W1b-W1a).T; b1]
    WLTD = nc.dram_tensor("WLTD", [65, 128], F16, kind="ExternalInput")   # [WLT | WLT]
    WLR = nc.dram_tensor("WLR", [65, 64], F16, kind="ExternalInput")      # [W2b.T; b2]
    WLS = nc.dram_tensor("WLS", [65, 64], F16, kind="ExternalInput")      # [W3b.T; b3]
    W2BLK = nc.dram_tensor("W2BLK", [128, 128], F16, kind="ExternalInput")
    W3ABLK = nc.dram_tensor("W3ABLK", [128, 128], F16, kind="ExternalInput")
    W3CBLK = nc.dram_tensor("W3CBLK", [128, 128], F16, kind="ExternalInput")
    EYE = nc.dram_tensor("EYE16", [128, 128], F16, kind="ExternalInput")
    EYE32 = nc.dram_tensor("EYE32", [128, 128], F32, kind="ExternalInput")
    Y = nc.dram_tensor("y", [C + 3 * G, N], F16, kind="ExternalOutput")

    PT_D = nc.dram_tensor("PT_D", [N, 64], F32, kind="Internal")
    IDXD = nc.dram_tensor("IDXD", [N * K], I16, kind="Internal")

    with TileContext(nc) as tc:
        with tc.tile_pool(name="const", bufs=1) as cp, \
             tc.tile_pool(name="work", bufs=2) as wp, \
             tc.tile_pool(name="chunk", bufs=1) as kp, \
             tc.tile_pool(name="gat", bufs=2) as gp, \
             tc.tile_pool(name="psS", bufs=2, space="PSUM") as ppsS, \
             tc.tile_pool(name="psA", bufs=1, space="PSUM") as ppsA, \
             tc.tile_pool(name="psU", bufs=1, space="PSUM") as ppsU, \
             tc.tile_pool(name="psC", bufs=2, space="PSUM") as ppsC:

            # ---------------- setup ----------------
            X65 = cp.tile([65, N], F32)
            X16 = cp.tile([65, N], F16)
            RHSB = cp.tile([128, N], F16)
            LHSB = cp.tile([128, N], F16)
            IOTAt = cp.tile([128, N], I32)
            PC = cp.tile([64, N], F32)
            TSTK = cp.tile([128, N], F16)
            RSTK = cp.tile([128, N], F16)
            SCt = cp.tile([64, N], F16)
            PTS = cp.tile([128, NT * 64], F32)
            IDXALL = cp.tile([128, NT * K], U16)
            EYE16 = cp.tile([128, 128], F16)
            EYE32t = cp.tile([128, 128], F32)
            wltp = cp.tile([64, 64], F16)
            wlt = cp.tile([65, 64], F16)
            wltd = cp.tile([65, 128], F16)
            wlr = cp.tile([65, 64], F16)
            wls = cp.tile([65, 64], F16)
            w2b = cp.tile([128, 128], F16)
            w3a = cp.tile([128, 128], F16)
            w3c = cp.tile([128, 128], F16)

            nc.sync.dma_start(out=X65[0:64, 0:1024], in_=x_in[:, 0:1024])
            nc.sync.dma_start(out=IOTAt[:, 0:1024], in_=IOTAI[:, 0:1024])
            nc.sync.dma_start(out=X65[0:64, 1024:2048], in_=x_in[:, 1024:2048])
            nc.sync.dma_start(out=IOTAt[:, 1024:2048], in_=IOTAI[:, 1024:2048])
            nc.sync.dma_start(out=EYE16[:, :], in_=EYE[:, :])
            nc.sync.dma_start(out=EYE32t[:, :], in_=EYE32[:, :])
            nc.sync.dma_start(out=wltp[:, :], in_=WLTP[:, :])
            nc.sync.dma_start(out=wlt[:, :], in_=WLT[:, :])
            nc.sync.dma_start(out=wltd[:, :], in_=WLTD[:, :])
            nc.sync.dma_start(out=wlr[:, :], in_=WLR[:, :])
            nc.sync.dma_start(out=wls[:, :], in_=WLS[:, :])
            nc.sync.dma_start(out=w2b[:, :], in_=W2BLK[:, :])
            nc.sync.dma_start(out=w3a[:, :], in_=W3ABLK[:, :])
            nc.sync.dma_start(out=w3c[:, :], in_=W3CBLK[:, :])
            nc.gpsimd.memset(X16[64:65, :], 1.0)
            nc.gpsimd.memset(LHSB[64:128, :], -1.0)

            # Startup conversions run on the (otherwise idle) DVE in halves so
            # tile 0's score matmuls start as soon as each x half lands.
            MUL = mybir.AluOpType.mult
            for half in range(2):
                hs = slice(half * 1024, (half + 1) * 1024)
                nc.vector.tensor_scalar(out=RHSB[0:64, hs], in0=X65[0:64, hs],
                                        scalar1=1.0, scalar2=None, op0=MUL)
                nc.vector.tensor_tensor(out=RHSB[64:128, hs], in0=X65[0:64, hs],
                                        in1=X65[0:64, hs], op=MUL)
                nc.vector.tensor_scalar(out=LHSB[0:64, hs], in0=X65[0:64, hs],
                                        scalar1=2.0, scalar2=None, op0=MUL)
                nc.vector.tensor_scalar(out=X16[0:64, hs], in0=X65[0:64, hs],
                                        scalar1=1.0, scalar2=None, op0=MUL)

            def emit_ptab(u):
                # P (c-layout) chunk u + its 4 P^T-table tiles; spread across
                # the first topk tile group so the ACT/PE work hides behind
                # the DVE-bound topk stream.
                sl = slice(u * 512, (u + 1) * 512)
                p1 = ppsU.tile([64, 512], F32, tag="u2")
                nc.tensor.matmul(p1[:, :], wltp[:, :], X16[0:64, sl], start=True, stop=True)
                nc.scalar.activation(PC[:, sl], p1[:, :], COPY)
                for rt in range(4 * u, 4 * u + 4):
                    pt = ppsA.tile([128, 512], F32, tag="a")
                    nc.tensor.transpose(pt[:, 0:64], PC[:, rt * 128:(rt + 1) * 128],
                                        EYE32t[0:64, 0:64])
                    nc.scalar.activation(PTS[:, rt * 64:(rt + 1) * 64], pt[:, 0:64], COPY)
                if u == 3:
                    nc.sync.dma_start(
                        out=AP(PT_D, 0, [[64, 128], [8192, NT], [1, 64]]),
                        in_=PTS[:, :].rearrange("p (a b) -> p a b", a=NT),
                    )
                    # x passthrough output rows 64:128 (fp16)
                    nc.sync.dma_start(out=Y[64:128, :], in_=X16[0:64, :])

            def emit_trs(u):
                # T/R stacked and S table chunk u; spread across the second
                # topk tile group (only needed by the first super-chunk).
                sl = slice(u * 512, (u + 1) * 512)
                p3 = ppsU.tile([64, 512], F32, tag="u2")
                nc.tensor.matmul(p3[:, :], wlr[:, :], X16[:, sl], start=True, stop=True)
                nc.scalar.activation(RSTK[0:64, sl], p3[:, :], COPY)
                p4 = ppsU.tile([64, 512], F32, tag="u2")
                nc.tensor.matmul(p4[:, :], wls[:, :], X16[:, sl], start=True, stop=True)
                nc.scalar.activation(SCt[:, sl], p4[:, :], COPY)
                nc.scalar.activation(RSTK[64:128, sl], RSTK[0:64, sl], COPY)

            # ---------------- scores + topk for one row tile ----------------
            # See module docstring: int32 (score<<12 | j) packing, fp32-view
            # max8 over eighths + exact merge, j = packed & 4095.
            def emit_topk(rt):
                if True:
                    PACKED = wp.tile([128, N], I32, tag="scores")
                    for u in range(4):
                        pss = ppsS.tile([128, 512], F32, tag="score")
                        nc.tensor.matmul(pss[:, :],
                                         LHSB[:, rt * 128:(rt + 1) * 128],
                                         RHSB[:, u * 512:(u + 1) * 512],
                                         start=True, stop=True)
                        nc.scalar.activation(PACKED[:, u * 512:(u + 1) * 512],
                                             pss[:, :], COPY,
                                             scale=512.0, bias=262144.0)
                    CAND = wp.tile([128, 64], F32, tag="cand")
                    T16 = wp.tile([128, 16], F32, tag="t16")
                    for half in range(2):
                        hs = slice(half * 1024, (half + 1) * 1024)
                        nc.vector.scalar_tensor_tensor(
                            out=PACKED[:, hs], in0=PACKED[:, hs], scalar=12,
                            in1=IOTAt[:, hs],
                            op0=mybir.AluOpType.logical_shift_left,
                            op1=mybir.AluOpType.bitwise_or)
                        for e in range(4 * half, 4 * half + 4):
                            nc.vector.max(out=CAND[:, 8 * e:8 * e + 8],
                                          in_=PACKED[:, 256 * e:256 * (e + 1)].bitcast(F32))
                    nc.vector.max(out=T16[:, 0:8], in_=CAND[:, :])
                    nc.vector.match_replace(out=CAND[:, :], in_to_replace=T16[:, 0:8],
                                            in_values=CAND[:, :], imm_value=0.0)
                    nc.vector.max(out=T16[:, 8:16], in_=CAND[:, :])
                    # bitvec ops need src/dst dtypes equal, so AND into i32
                    # then convert to u16 with an arithmetic op.
                    T16I = wp.tile([128, 16], I32, tag="t16i")
                    nc.vector.tensor_scalar(out=T16I[:, :],
                                            in0=T16[:, :].bitcast(I32),
                                            scalar1=4095, scalar2=None,
                                            op0=mybir.AluOpType.bitwise_and)
                    nc.vector.tensor_scalar(out=IDXALL[:, rt * K:(rt + 1) * K],
                                            in0=T16I[:, :],
                                            scalar1=0, scalar2=None,
                                            op0=mybir.AluOpType.add)

            # ---------------- per node-tile gather + edge MLP block ----------
            # One gather per 128-node tile (2048 edges): the idx chain starts
            # right after the tile's own topk extract and the 4x-smaller
            # transfer pipelines tile-by-tile instead of serializing 12us
            # slabs on the DMA track.
            RED = {}
            BLK = {}

            PGD = {}

            def emit_gather(pr):
                # One gather per PAIR of node tiles (4096 edges): amortizes
                # the idx write/read/replication hop latency over two tiles.
                # IDXD layout: addr = pr*4096 + r*32 + t*16 + k (t = tile
                # parity; contiguous 64B runs per partition on the write).
                nc.sync.dma_start(
                    out=AP(IDXD, pr * 4096, [[32, 128], [16, 2], [1, K]]),
                    in_=IDXALL[:, 2 * pr * K:(2 * pr + 2) * K].bitcast(I16)
                        .rearrange("p (t k) -> p t k", t=2),
                )
                idxt = gp.tile([128, 256], I16, tag="idxt")
                # idxt[g*16+m, s'], s' = j'*8 + c, j' = t*16 + j:
                #   <- addr pr*4096 + (m+16c)*32 + t*16 + j
                src_w = AP(IDXD, pr * 4096,
                           [[32, 16], [16, 2], [1, K], [512, 8]])
                nc.sync.dma_start(out=idxt[0:16, :], in_=src_w)
                for lo, hi in ((16, 32), (32, 64), (64, 128)):
                    nc.sync.dma_start(out=idxt[lo:hi, :], in_=idxt[0:lo, :])

                pgnew = gp.tile([128, 2 * K, 64], F32, tag="pg")
                PGD[2 * pr] = (pgnew, 0)
                PGD[2 * pr + 1] = (pgnew, K)
                nc.gpsimd.dma_gather(
                    out_ap=pgnew[:, :, :], in_ap=PT_D.ap(), idxs_ap=idxt[:, :],
                    num_idxs=4096, num_idxs_reg=4096, elem_size=64,
                    single_packet=False,
                )

            def emit_gather_single(rt):
                # Single-tile gather (2048 edges) for the LAST two tiles: the
                # tail cannot hide a pair chain behind later topk work, so
                # tile 14's block starts as soon as its own indices exist.
                # IDXD layout: addr = rt*2048 + r*16 + k.
                nc.sync.dma_start(
                    out=AP(IDXD, rt * 2048, [[16, 128], [1, K]]),
                    in_=IDXALL[:, rt * K:(rt + 1) * K].bitcast(I16),
                )
                idxt = gp.tile([128, 256], I16, tag="idxt")
                # idxt[g*16+m, s'], s' = j*8 + c <- addr rt*2048 + (m+16c)*16 + j
                src_w = AP(IDXD, rt * 2048, [[16, 16], [1, K], [256, 8]])
                nc.sync.dma_start(out=idxt[0:16, 0:128], in_=src_w)
                for lo, hi in ((16, 32), (32, 64), (64, 128)):
                    nc.sync.dma_start(out=idxt[lo:hi, 0:128], in_=idxt[0:lo, 0:128])

                pgnew = gp.tile([128, 2 * K, 64], F32, tag="pg")
                PGD[rt] = (pgnew, 0)
                nc.gpsimd.dma_gather(
                    out_ap=pgnew[:, 0:K, :], in_ap=PT_D.ap(),
                    idxs_ap=idxt[:, 0:128],
                    num_idxs=2048, num_idxs_reg=2048, elem_size=64,
                    single_packet=False,
                )

            def emit_block(rt):
                sc, bl = rt // 4, rt % 4
                pgt, joff = PGD.pop(rt)
                PG = pgt[:, joff:joff + K, :]

                AC = kp.tile([128, 8, 128], F16, tag=f"ac{rt % 2}")
                B2C = kp.tile([128, 8, 128], F16, tag=f"b2c{rt % 2}")
                rb = RSTK[:, rt * 128:(rt + 1) * 128].unsqueeze(1).broadcast_to([128, 4, 128])
                # T^T for this tile's nodes, computed directly (one matmul:
                # X16-block^T x [wlt|wlt]); accumulated into each psa block
                # as a second is_transpose matmul so the T bias lands in the
                # conv1 PSUM and the 19us DVE add disappears — ACT's relu
                # reads the psum directly.
                ttp_ps = ppsU.tile([128, 128], F32, tag="u2")
                nc.tensor.matmul(ttp_ps[:, :], X16[:, rt * 128:(rt + 1) * 128],
                                 wltd[:, :], start=True, stop=True)
                ttp = kp.tile([128, 128], F32, tag=f"ttp{rt % 2}")
                nc.scalar.activation(ttp[:, :], ttp_ps[:, :], COPY)
                for q in range(2):
                    # transposes: 4 kp blocks + T^T -> psum (128, 512)
                    psa = ppsA.tile([128, 512], F32, tag="a")
                    for kk in range(4):
                        kpi = q * 4 + kk
                        blk = PG[:, 2 * kpi:2 * kpi + 2, :]
                        nc.tensor.matmul(psa[:, kk * 128:(kk + 1) * 128],
                                         blk, EYE32t[:, :], is_transpose=True,
                                         start=True, stop=False,
                                         skip_group_check=True)
                        nc.tensor.matmul(psa[:, kk * 128:(kk + 1) * 128],
                                         ttp[:, :], EYE32t[:, :], is_transpose=True,
                                         start=False, stop=True,
                                         skip_group_check=True)
                    nc.scalar.activation(AC[:, 4 * q:4 * q + 4, :],
                                         psa[:, :].rearrange("p (a b) -> p a b", a=4),
                                         RELU)

                    # conv2 (+R folded in as an identity-matmul accumulate)
                    ps2t = ppsU.tile([128, 512], F32, tag="u2")
                    nc.tensor.matmul(ps2t[:, :], w2b[:, :],
                                     AC[:, 4 * q:4 * q + 4, :],
                                     start=True, stop=False,
                                     skip_group_check=True)
                    nc.tensor.matmul(ps2t[:, :].rearrange("p (a b) -> p a b", a=4),
                                     EYE16[:, :], rb,
                                     start=False, stop=True,
                                     skip_group_check=True)
                    nc.scalar.activation(B2C[:, 4 * q:4 * q + 4, :], ps2t[:, :], RELU)

                # conv3 for both halves into one psum tile; first k-max level
                # (k pairs 4 apart) reads the psum halves directly.
                psc = ppsC.tile([128, 1024], F32, tag="c3")
                for q in range(2):
                    nc.tensor.matmul(psc[:, q * 512:(q + 1) * 512], w3a[:, :],
                                     AC[:, 4 * q:4 * q + 4, :],
                                     start=True, stop=False)
                    nc.tensor.matmul(psc[:, q * 512:(q + 1) * 512], w3c[:, :],
                                     B2C[:, 4 * q:4 * q + 4, :],
                                     start=False, stop=True)
                # DVE may read only one PSUM operand: evacuate the q1 half so
                # the C3 first-level max pairs psum against SBUF.
                c3h = kp.tile([128, 4, 128], F16, tag=f"c3h{rt % 2}")
                nc.scalar.activation(c3h[:, :, :],
                                     psc[:, 512:1024].rearrange("p (a b) -> p a b", a=4),
                                     COPY)
                BLK[rt] = (AC, B2C, psc, c3h)

            def emit_trees(rt):
                # k-max trees (fp16, 2x DVE), deferred one tile so every
                # input (relu evacs, conv3 psum) is long done when the DVE
                # stream reaches them — no cross-engine ping-pong stalls.
                sc, bl = rt // 4, rt % 4
                AC, B2C, psc, c3h = BLK.pop(rt)
                for (src, row0, lv) in ((AC, 0, 3), (B2C, 2 * G, 3), (psc, 3 * G, 2)):
                    if bl == 0:
                        rednew = kp.tile([128, NBL, 128], F16,
                                         tag=f"red{row0}{sc % 2}")
                        RED[(sc, row0)] = rednew
                    red = RED[(sc, row0)]
                    if lv == 3:
                        m1 = kp.tile([128, 4, 128], F16, tag=f"m1{row0}")
                        nc.vector.tensor_tensor(out=m1[:, :, :], in0=src[:, 0:4, :],
                                                in1=src[:, 4:8, :], op=MAX)
                    else:
                        m1 = kp.tile([128, 4, 128], F16, tag=f"m1{row0}")
                        nc.vector.tensor_tensor(
                            out=m1[:, :, :],
                            in0=src[:, 0:512].rearrange("p (a b) -> p a b", a=4),
                            in1=c3h[:, :, :],
                            op=MAX)
                    m2 = kp.tile([128, 2, 128], F16, tag=f"m2{row0}")
                    nc.vector.tensor_tensor(out=m2[:, :, :], in0=m1[:, 0:2, :],
                                            in1=m1[:, 2:4, :], op=MAX)
                    nc.vector.tensor_tensor(out=red[:, bl, :], in0=m2[:, 0, :],
                                            in1=m2[:, 1, :], op=MAX)

            def emit_om(sc):
                # cross-half merge (4x stt) + S bias + output DMAs, per sc.
                for (row0, add_s) in ((0, False), (2 * G, False), (3 * G, True)):
                    red = RED[(sc, row0)]
                    hi = kp.tile([64, NBL * 128], F16, tag=f"hi{row0}")
                    nc.scalar.activation(hi[:, :],
                                         red[64:128, :, :].rearrange("p a n -> p (a n)"),
                                         COPY)
                    om = kp.tile([64, NBL * 128], F16, tag=f"om{row0}")
                    nc.vector.tensor_tensor(
                        out=om[:, :],
                        in0=red[0:64, :, :].rearrange("p a n -> p (a n)"),
                        in1=hi[:, :], op=MAX)
                    if add_s:
                        om2 = kp.tile([64, NBL * 128], F16, tag="oms")
                        nc.vector.tensor_tensor(out=om2[:, :], in0=om[:, :],
                                                in1=SCt[:, sc * 512:(sc + 1) * 512],
                                                op=ADD)
                        om = om2
                    nc.sync.dma_start(out=Y[row0 if row0 else 0:(row0 if row0 else 0) + 64,
                                            sc * 512:(sc + 1) * 512],
                                      in_=om[:, :])

            # Interleaved emission: engines execute their streams in emission
            # order, so super-chunk work is placed one tile-group behind the
            # topk tiles whose indices it needs — sc_i's gather round-trip
            # hides behind tile group i+1's topk, and the table setup spreads
            # across groups 0 (P table) and 1 (T/R/S tables).
            for rt in range(NT):
                emit_topk(rt)
                # P table over tiles 0-2 (PT_D written before the first
                # gather's idx DMAs in queue order); T/R/S chunk u at tile 4u
                # (chunk u is first needed by block 4u at iteration 4u+3).
                if rt == 0:
                    emit_ptab(0)
                elif rt == 1:
                    emit_ptab(1)
                    emit_ptab(2)
                elif rt == 2:
                    emit_ptab(3)
                # pair gathers; pr=0 is deferred one tile so the PT_D table
                # write (ptab(3), tile 2) precedes it in DMA-queue order.
                if rt == 2:
                    emit_gather(0)
                elif rt % 2 == 1 and 3 <= rt <= 13:
                    emit_gather(rt // 2)
                elif rt >= 14:
                    emit_gather_single(rt)
                if rt >= 5:
                    emit_block(rt - 5)
                if rt >= 6:
                    emit_trees(rt - 6)
                    if (rt - 6) % 4 == 3:
                        emit_om((rt - 6) // 4)
                # T/R/S after block work: the PE reaches the DVE-blocking
                # transposes before burning time on the tables.
                if rt % 4 == 0:
                    emit_trs(rt // 4)
            for br in (NT - 5, NT - 4, NT - 3, NT - 2, NT - 1):
                emit_block(br)
                emit_trees(br - 1)
                if (br - 1) % 4 == 3:
                    emit_om((br - 1) // 4)
            emit_trees(NT - 1)
            emit_om(3)

    _fix_int_imms(nc)
    _split_all_waits(nc)
    _insert_gpsimd_library_load(nc, 3)
    return nc


def _prep_weights(W1, b1, W2, b2, W3, b3):
    W1 = np.asarray(W1, np.float32); W2 = np.asarray(W2, np.float32)
    W3 = np.asarray(W3, np.float32)
    b1 = np.asarray(b1, np.float32); b2 = np.asarray(b2, np.float32)
    b3 = np.asarray(b3, np.float32)
    W1a, W1b = W1[:, :64], W1[:, 64:]
    W2a, W2b = W2[:, :64], W2[:, 64:]
    W3a, W3b, W3c = W3[:, :64], W3[:, 64:128], W3[:, 128:]

    def blk(w):
        z = np.zeros((128, 128), np.float16)
        z[0:64, 0:64] = w.T
        z[64:128, 64:128] = w.T
        return z

    f16 = np.float16
    return {
        "WLTP": np.ascontiguousarray(W1a.T).astype(f16),
        "WLT": np.ascontiguousarray(np.vstack([(W1b - W1a).T, b1[None, :]])).astype(f16),
        "WLTD": np.ascontiguousarray(np.tile(np.vstack([(W1b - W1a).T, b1[None, :]]), (1, 2))).astype(f16),
        "WLR": np.ascontiguousarray(np.vstack([W2b.T, b2[None, :]])).astype(f16),
        "WLS": np.ascontiguousarray(np.vstack([W3b.T, b3[None, :]])).astype(f16),
        "W2BLK": blk(W2a),
        "W3ABLK": blk(W3a),
        "W3CBLK": blk(W3c),
        "EYE16": np.eye(128, dtype=f16),
        "EYE32": np.eye(128, dtype=np.float32),
        "IOTAI": np.tile(np.arange(N, dtype=np.int32), (128, 1)),
    }


_NC = None


def kernel(x, W1, b1, W2, b2, W3, b3):
    global _NC
    if _NC is None:
        _NC = build()
    x = np.asarray(x, np.float32)
    w = _prep_weights(W1, b1, W2, b2, W3, b3)
    in_maps = [{"x": np.ascontiguousarray(x[b]), **w} for b in range(B)]
    res = run_bass_kernel_spmd(_NC, in_maps, core_ids=list(range(B)))
    return np.stack([res.results[b]["y"].astype(np.float32) for b in range(B)], axis=0)


# revision 118
# speedup vs baseline: 1.1357x; 1.0408x over previous
"""DGCNN-style edge-conv block (KNN graph + dense conv stack) on 8 trn2 cores.

Strategy (data-parallel over batch, one batch element per core):
  scores   = 2<xi,xj> - ||xj||^2 via one fp16 PE matmul with [2x; -1] x [x; x^2]
             contraction (the -||xi||^2 term is a per-row constant and cannot
             change a row's top-k, so it is dropped).
  top-16   = int32 bit-packing: ACT evacuates q = int32(psum*512 + 2^18)
             (positive 19-bit), DVE packs (q << 12) | j. Non-negative int32
             bit patterns order identically under an fp32 view, so max8 /
             match_replace on the bitcast yield values AND indices
             (j = packed & 4095). Top-16 = 8x max8 over disjoint 256-wide
             eighths + exact 64-wide merge (max8 / match_replace / max8);
             only rows with >8 of their true top-16 in one eighth (~1e-4 of
             rows) can lose a tail neighbor.
  gather   = P^T table (P = W1a @ x, 64 ch fp16 = 128B rows) in DRAM,
             gathered per 8192-edge super-chunk with gpsimd dma_gather
             (mlp ucode library, single_packet=False).
  edge MLP = A = relu(P_j + T_n), B2 = relu(W2a A + R_n),
             C3 = W3a A + W3c B2 + S_n, with T/R/S = per-node tables from
             small fp16 matmuls; per-edge convs run as fp16 block-diag
             matmuls on PE with 2k-stacked PE transposes. The R bias is
             folded into the conv2 PSUM group as an identity x R-broadcast
             fp16 matmul. C3 is never evacuated: its k-max first level reads
             the two conv3 PSUM halves directly.
  output   = channel-concat [max_k A; x; max_k B2; max_k C3] in fp16
             (host upconverts to fp32); k-max trees run as fp16
             tensor_tensor trees (2x DVE mode) with the cross-half merge
             fused into one scalar_tensor_tensor (4x mode).

Schedule: all 16 row-tiles' scores+topk are emitted first; each super-chunk's
gather/transpose/conv/max stages trail behind on DMA, PE and ACT as soon as
its 4 index tiles are ready.
"""

import numpy as np

import bass_rust
import concourse.bass as bass
import concourse.bass_isa as bass_isa
import concourse.mybir as mybir
from concourse.bass_types import AP
from concourse.tile import TileContext
from concourse.bass_utils import run_bass_kernel_spmd

F32 = mybir.dt.float32
F16 = mybir.dt.float16
I32 = mybir.dt.int32
U16 = mybir.dt.uint16
I16 = mybir.dt.int16

B, C, N, K, G = 8, 64, 2048, 16, 64
NT = 16          # 128-row tiles
NSC = 4          # super-chunks
NBL = 4          # nblocks per super-chunk
RELU = mybir.ActivationFunctionType.Relu
COPY = mybir.ActivationFunctionType.Copy
SQUARE = mybir.ActivationFunctionType.Square
ADD = mybir.AluOpType.add
MAX = mybir.AluOpType.max

_nop_ctr = [0]


def _split_all_waits(nc, max_waits=1):
    # This walrus build rejects >1 sync-wait on several CTRL structs; hoist
    # extras onto single-wait NOPs placed just before the instruction.
    for fn in nc.m.functions:
        for bb in fn.blocks:
            out = []
            for ins in bb.instructions:
                si = ins.sync_info
                if si is not None and si.on_wait is not None and len(si.on_wait) > max_waits:
                    waits = list(si.on_wait)
                    for w in waits[:-max_waits]:
                        _nop_ctr[0] += 1
                        nop = mybir.InstNoOp(name=f"waitnop-{_nop_ctr[0]}", ins=[], outs=[])
                        nop.engine = ins.engine
                        nop.sync_info = bass_rust.SyncInfo(on_wait=[w], on_update=[])
                        out.append(nop)
                        nc.register_instruction(nop, overwrite=True)
                    si.on_wait = waits[-max_waits:]
                out.append(ins)
            bb.instructions = out


def _fix_int_imms(nc):
    # walrus requires bitvec-op immediates to be integer-typed and match the
    # src/dst dtype; bass lowers python ints to float32 ImmVals, so retype
    # the immediates on int32 shift/bitwise TensorScalarPtr ops.
    bitvec = (mybir.AluOpType.logical_shift_left,
              mybir.AluOpType.logical_shift_right,
              mybir.AluOpType.arith_shift_left,
              mybir.AluOpType.arith_shift_right,
              mybir.AluOpType.bitwise_and,
              mybir.AluOpType.bitwise_or,
              mybir.AluOpType.bitwise_xor)
    for fn in nc.m.functions:
        for bb in fn.blocks:
            for ins in bb.instructions:
                if not isinstance(ins, mybir.InstTensorScalarPtr):
                    continue
                if ins.op0 not in bitvec and getattr(ins, "op1", None) not in bitvec:
                    continue
                new_ins = list(ins.ins)
                changed = False
                for i, a in enumerate(new_ins):
                    if isinstance(a, mybir.ImmediateValue) and a.dtype != mybir.dt.int32:
                        new_ins[i] = mybir.ImmediateValue(dtype=mybir.dt.int32,
                                                          value=int(a.value))
                        changed = True
                if changed:
                    ins.ins = new_ins


def _insert_gpsimd_library_load(nc, lib_index=3):
    # InstDMAGatherAnt needs the 'mlp' GPSIMD ucode library; raw Bass+Tile
    # skips Bacc's insert_library_loads, so prepend the reload by hand.
    ins = bass_isa.InstPseudoReloadLibraryIndex(
        name="libload-manual", ins=[], outs=[], lib_index=lib_index
    )
    ins.engine = mybir.EngineType.Pool
    nc.register_instruction(ins, overwrite=True)
    bb0 = nc.m.functions[0].blocks[0]
    bb0.instructions = [ins] + list(bb0.instructions)
    mybir.codegen_inst_isa_subclasses(nc)


def build():
    nc = bass.Bass("TRN2", debug=False, num_devices=8)

    x_in = nc.dram_tensor("x", [C, N], F32, kind="ExternalInput")
    IOTAI = nc.dram_tensor("IOTAI", [128, N], I32, kind="ExternalInput")
    WLTP = nc.dram_tensor("WLTP", [64, 64], F16, kind="ExternalInput")    # W1a.T
    WLT = nc.dram_tensor("WLT", [65, 64], F16, kind="ExternalInput")      # [(W1b-W1a).T; b1]
    WLTD = nc.dram_tensor("WLTD", [65, 128], F16, kind="ExternalInput")   # [WLT | WLT]
    WLR = nc.dram_tensor("WLR", [65, 64], F16, kind="ExternalInput")      # [W2b.T; b2]
    WLS = nc.dram_tensor("WLS", [65, 64], F16, kind="ExternalInput")      # [W3b.T; b3]
    W2BLK = nc.dram_tensor("W2BLK", [128, 128], F16, kind="ExternalInput")
    W3ABLK = nc.dram_tensor("W3ABLK", [128, 128], F16, kind="ExternalInput")
    W3CBLK = nc.dram_tensor("W3CBLK", [128, 128], F16, kind="ExternalInput")
    EYE = nc.dram_tensor("EYE16", [128, 128], F16, kind="ExternalInput")
    EYE32 = nc.dram_tensor("EYE32", [128, 128], F32, kind="ExternalInput")
    Y = nc.dram_tensor("y", [C + 3 * G, N], F16, kind="ExternalOutput")

    PT_D = nc.dram_tensor("PT_D", [N, 64], F32, kind="Internal")
    IDXD = nc.dram_tensor("IDXD", [N * K], I16, kind="Internal")

    with TileContext(nc) as tc:
        with tc.tile_pool(name="const", bufs=1) as cp, \
             tc.tile_pool(name="work", bufs=2) as wp, \
             tc.tile_pool(name="chunk", bufs=1) as kp, \
             tc.tile_pool(name="gat", bufs=2) as gp, \
             tc.tile_pool(name="psS", bufs=2, space="PSUM") as ppsS, \
             tc.tile_pool(name="psA", bufs=1, space="PSUM") as ppsA, \
             tc.tile_pool(name="psU", bufs=1, space="PSUM") as ppsU, \
             tc.tile_pool(name="psC", bufs=2, space="PSUM") as ppsC:

            # ---------------- setup ----------------
            X65 = cp.tile([65, N], F32)
            X16 = cp.tile([65, N], F16)
            RHSB = cp.tile([128, N], F16)
            LHSB = cp.tile([128, N], F16)
            IOTAt = cp.tile([128, N], I32)
            PC = cp.tile([64, N], F32)
            TSTK = cp.tile([128, N], F16)
            RSTK = cp.tile([128, N], F16)
            SCt = cp.tile([64, N], F16)
            PTS = cp.tile([128, NT * 64], F32)
            IDXALL = cp.tile([128, NT * K], U16)
            EYE16 = cp.tile([128, 128], F16)
            EYE32t = cp.tile([128, 128], F32)
            wltp = cp.tile([64, 64], F16)
            wlt = cp.tile([65, 64], F16)
            wltd = cp.tile([65, 128], F16)
            wlr = cp.tile([65, 64], F16)
            wls = cp.tile([65, 64], F16)
            w2b = cp.tile([128, 128], F16)
            w3a = cp.tile([128, 128], F16)
            w3c = cp.tile([128, 128], F16)

            nc.sync.dma_start(out=X65[0:64, 0:1024], in_=x_in[:, 0:1024])
            nc.sync.dma_start(out=IOTAt[:, 0:1024], in_=IOTAI[:, 0:1024])
            nc.sync.dma_start(out=X65[0:64, 1024:2048], in_=x_in[:, 1024:2048])
            nc.sync.dma_start(out=IOTAt[:, 1024:2048], in_=IOTAI[:, 1024:2048])
            nc.sync.dma_start(out=EYE16[:, :], in_=EYE[:, :])
            nc.sync.dma_start(out=EYE32t[:, :], in_=EYE32[:, :])
            nc.sync.dma_start(out=wltp[:, :], in_=WLTP[:, :])
            nc.sync.dma_start(out=wlt[:, :], in_=WLT[:, :])
            nc.sync.dma_start(out=wltd[:, :], in_=WLTD[:, :])
            nc.sync.dma_start(out=wlr[:, :], in_=WLR[:, :])
            nc.sync.dma_start(out=wls[:, :], in_=WLS[:, :])
            nc.sync.dma_start(out=w2b[:, :], in_=W2BLK[:, :])
            nc.sync.dma_start(out=w3a[:, :], in_=W3ABLK[:, :])
            nc.sync.dma_start(out=w3c[:, :], in_=W3CBLK[:, :])
            nc.gpsimd.memset(X16[64:65, :], 1.0)
            nc.gpsimd.memset(LHSB[64:128, :], -1.0)

            # Startup conversions run on the (otherwise idle) DVE in halves so
            # tile 0's score matmuls start as soon as each x half lands.
            MUL = mybir.AluOpType.mult
            for half in range(2):
                hs = slice(half * 1024, (half + 1) * 1024)
                nc.vector.tensor_scalar(out=RHSB[0:64, hs], in0=X65[0:64, hs],
                                        scalar1=1.0, scalar2=None, op0=MUL)
                nc.vector.tensor_tensor(out=RHSB[64:128, hs], in0=X65[0:64, hs],
                                        in1=X65[0:64, hs], op=MUL)
                nc.vector.tensor_scalar(out=LHSB[0:64, hs], in0=X65[0:64, hs],
                                        scalar1=2.0, scalar2=None, op0=MUL)
                nc.vector.tensor_scalar(out=X16[0:64, hs], in0=X65[0:64, hs],
                                        scalar1=1.0, scalar2=None, op0=MUL)

            def emit_ptab(u):
                # P (c-layout) chunk u + its 4 P^T-table tiles; spread across
                # the first topk tile group so the ACT/PE work hides behind
                # the DVE-bound topk stream.
                sl = slice(u * 512, (u + 1) * 512)
                p1 = ppsU.tile([64, 512], F32, tag="u2")
                nc.tensor.matmul(p1[:, :], wltp[:, :], X16[0:64, sl], start=True, stop=True)
                nc.scalar.activation(PC[:, sl], p1[:, :], COPY)
                for rt in range(4 * u, 4 * u + 4):
                    pt = ppsA.tile([128, 512], F32, tag="a")
                    nc.tensor.transpose(pt[:, 0:64], PC[:, rt * 128:(rt + 1) * 128],
                                        EYE32t[0:64, 0:64])
                    nc.scalar.activation(PTS[:, rt * 64:(rt + 1) * 64], pt[:, 0:64], COPY)
                if u == 3:
                    nc.sync.dma_start(
                        out=AP(PT_D, 0, [[64, 128], [8192, NT], [1, 64]]),
                        in_=PTS[:, :].rearrange("p (a b) -> p a b", a=NT),
                    )
                    # x passthrough output rows 64:128 (fp16)
                    nc.sync.dma_start(out=Y[64:128, :], in_=X16[0:64, :])

            def emit_trs(u):
                # T/R stacked and S table chunk u; spread across the second
                # topk tile group (only needed by the first super-chunk).
                sl = slice(u * 512, (u + 1) * 512)
                p3 = ppsU.tile([64, 512], F32, tag="u2")
                nc.tensor.matmul(p3[:, :], wlr[:, :], X16[:, sl], start=True, stop=True)
                nc.scalar.activation(RSTK[0:64, sl], p3[:, :], COPY)
                p4 = ppsU.tile([64, 512], F32, tag="u2")
                nc.tensor.matmul(p4[:, :], wls[:, :], X16[:, sl], start=True, stop=True)
                nc.scalar.activation(SCt[:, sl], p4[:, :], COPY)
                nc.scalar.activation(RSTK[64:128, sl], RSTK[0:64, sl], COPY)

            # ---------------- scores + topk for one row tile ----------------
            # See module docstring: int32 (score<<12 | j) packing, fp32-view
            # max8 over eighths + exact merge, j = packed & 4095.
            def emit_topk(rt):
                if True:
                    PACKED = wp.tile([128, N], I32, tag="scores")
                    for u in range(4):
                        pss = ppsS.tile([128, 512], F32, tag="score")
                        nc.tensor.matmul(pss[:, :],
                                         LHSB[:, rt * 128:(rt + 1) * 128],
                                         RHSB[:, u * 512:(u + 1) * 512],
                                         start=True, stop=True)
                        nc.scalar.activation(PACKED[:, u * 512:(u + 1) * 512],
                                             pss[:, :], COPY,
                                             scale=512.0, bias=262144.0)
                    CAND = wp.tile([128, 64], F32, tag="cand")
                    T16 = wp.tile([128, 16], F32, tag="t16")
                    for half in range(2):
                        hs = slice(half * 1024, (half + 1) * 1024)
                        nc.vector.scalar_tensor_tensor(
                            out=PACKED[:, hs], in0=PACKED[:, hs], scalar=12,
                            in1=IOTAt[:, hs],
                            op0=mybir.AluOpType.logical_shift_left,
                            op1=mybir.AluOpType.bitwise_or)
                        for e in range(4 * half, 4 * half + 4):
                            nc.vector.max(out=CAND[:, 8 * e:8 * e + 8],
                                          in_=PACKED[:, 256 * e:256 * (e + 1)].bitcast(F32))
                    nc.vector.max(out=T16[:, 0:8], in_=CAND[:, :])
                    nc.vector.match_replace(out=CAND[:, :], in_to_replace=T16[:, 0:8],
                                            in_values=CAND[:, :], imm_value=0.0)
                    nc.vector.max(out=T16[:, 8:16], in_=CAND[:, :])
                    # bitvec ops need src/dst dtypes equal, so AND into i32
                    # then convert to u16 with an arithmetic op.
                    T16I = wp.tile([128, 16], I32, tag="t16i")
                    nc.vector.tensor_scalar(out=T16I[:, :],
                                            in0=T16[:, :].bitcast(I32),
                                            scalar1=4095, scalar2=None,
                                            op0=mybir.AluOpType.bitwise_and)
                    nc.vector.tensor_scalar(out=IDXALL[:, rt * K:(rt + 1) * K],
                                            in0=T16I[:, :],
                                            scalar1=0, scalar2=None,
                                            op0=mybir.AluOpType.add)

            # ---------------- per node-tile gather + edge MLP block ----------
            # One gather per 128-node tile (2048 edges): the idx chain starts
            # right after the tile's own topk extract and the 4x-smaller
            # transfer pipelines tile-by-tile instead of serializing 12us
            # slabs on the DMA track.
            RED = {}
            BLK = {}

            PGD = {}

            def emit_gather(pr):
                # One gather per PAIR of node tiles (4096 edges): amortizes
                # the idx write/read/replication hop latency over two tiles.
                # IDXD layout: addr = pr*4096 + r*32 + t*16 + k (t = tile
                # parity; contiguous 64B runs per partition on the write).
                nc.sync.dma_start(
                    out=AP(IDXD, pr * 4096, [[32, 128], [16, 2], [1, K]]),
                    in_=IDXALL[:, 2 * pr * K:(2 * pr + 2) * K].bitcast(I16)
                        .rearrange("p (t k) -> p t k", t=2),
                )
                idxt = gp.tile([128, 256], I16, tag="idxt")
                # idxt[g*16+m, s'], s' = j'*8 + c, j' = t*16 + j:
                #   <- addr pr*4096 + (m+16c)*32 + t*16 + j
                src_w = AP(IDXD, pr * 4096,
                           [[32, 16], [16, 2], [1, K], [512, 8]])
                nc.sync.dma_start(out=idxt[0:16, :], in_=src_w)
                for lo, hi in ((16, 32), (32, 64), (64, 128)):
                    nc.sync.dma_start(out=idxt[lo:hi, :], in_=idxt[0:lo, :])

                pgnew = gp.tile([128, 2 * K, 64], F32, tag="pg")
                PGD[2 * pr] = (pgnew, 0)
                PGD[2 * pr + 1] = (pgnew, K)
                nc.gpsimd.dma_gather(
                    out_ap=pgnew[:, :, :], in_ap=PT_D.ap(), idxs_ap=idxt[:, :],
                    num_idxs=4096, num_idxs_reg=4096, elem_size=64,
                    single_packet=False,
                )

            def emit_gather_single(rt):
                # Single-tile gather (2048 edges) for the LAST two tiles: the
                # tail cannot hide a pair chain behind later topk work, so
                # tile 14's block starts as soon as its own indices exist.
                # IDXD layout: addr = rt*2048 + r*16 + k.
                nc.sync.dma_start(
                    out=AP(IDXD, rt * 2048, [[16, 128], [1, K]]),
                    in_=IDXALL[:, rt * K:(rt + 1) * K].bitcast(I16),
                )
                idxt = gp.tile([128, 256], I16, tag="idxt")
                # idxt[g*16+m, s'], s' = j*8 + c <- addr rt*2048 + (m+16c)*16 + j
                src_w = AP(IDXD, rt * 2048, [[16, 16], [1, K], [256, 8]])
                nc.sync.dma_start(out=idxt[0:16, 0:128], in_=src_w)
                for lo, hi in ((16, 32), (32, 64), (64, 128)):
                    nc.sync.dma_start(out=idxt[lo:hi, 0:128], in_=idxt[0:lo, 0:128])

                pgnew = gp.tile([128, 2 * K, 64], F32, tag="pg")
                PGD[rt] = (pgnew, 0)
                nc.gpsimd.dma_gather(
                    out_ap=pgnew[:, 0:K, :], in_ap=PT_D.ap(),
                    idxs_ap=idxt[:, 0:128],
                    num_idxs=2048, num_idxs_reg=2048, elem_size=64,
                    single_packet=False,
                )

            def emit_block(rt):
                sc, bl = rt // 4, rt % 4
                pgt, joff = PGD.pop(rt)
                PG = pgt[:, joff:joff + K, :]

                AC = kp.tile([128, 8, 128], F16, tag=f"ac{rt % 2}")
                B2C = kp.tile([128, 8, 128], F16, tag=f"b2c{rt % 2}")
                rb = RSTK[:, rt * 128:(rt + 1) * 128].unsqueeze(1).broadcast_to([128, 4, 128])
                # T^T for this tile's nodes, computed directly (one matmul:
                # X16-block^T x [wlt|wlt]); accumulated into each psa block
                # as a second is_transpose matmul so the T bias lands in the
                # conv1 PSUM and the 19us DVE add disappears — ACT's relu
                # reads the psum directly.
                ttp_ps = ppsU.tile([128, 128], F32, tag="u2")
                nc.tensor.matmul(ttp_ps[:, :], X16[:, rt * 128:(rt + 1) * 128],
                                 wltd[:, :], start=True, stop=True)
                ttp = kp.tile([128, 128], F32, tag=f"ttp{rt % 2}")
                nc.scalar.activation(ttp[:, :], ttp_ps[:, :], COPY)
                for q in range(2):
                    # transposes: 4 kp blocks + T^T -> psum (128, 512)
                    psa = ppsA.tile([128, 512], F32, tag="a")
                    for kk in range(4):
                        kpi = q * 4 + kk
                        blk = PG[:, 2 * kpi:2 * kpi + 2, :]
                        nc.tensor.matmul(psa[:, kk * 128:(kk + 1) * 128],
                                         blk, EYE32t[:, :], is_transpose=True,
                                         start=True, stop=False,
                                         skip_group_check=True)
                        nc.tensor.matmul(psa[:, kk * 128:(kk + 1) * 128],
                                         ttp[:, :], EYE32t[:, :], is_transpose=True,
                                         start=False, stop=True,
                                         skip_group_check=True)
                    nc.scalar.activation(AC[:, 4 * q:4 * q + 4, :],
                                         psa[:, :].rearrange("p (a b) -> p a b", a=4),
                                         RELU)

                    # conv2 (+R folded in as an identity-matmul accumulate)
                    ps2t = ppsU.tile([128, 512], F32, tag="u2")
                    nc.tensor.matmul(ps2t[:, :], w2b[:, :],
                                     AC[:, 4 * q:4 * q + 4, :],
                                     start=True, stop=False,
                                     skip_group_check=True)
                    nc.tensor.matmul(ps2t[:, :].rearrange("p (a b) -> p a b", a=4),
                                     EYE16[:, :], rb,
                                     start=False, stop=True,
                                     skip_group_check=True)
                    nc.scalar.activation(B2C[:, 4 * q:4 * q + 4, :], ps2t[:, :], RELU)

                # conv3 for both halves into one psum tile; first k-max level
                # (k pairs 4 apart) reads the psum halves directly.
                psc = ppsC.tile([128, 1024], F32, tag="c3")
                for q in range(2):
                    nc.tensor.matmul(psc[:, q * 512:(q + 1) * 512], w3a[:, :],
                                     AC[:, 4 * q:4 * q + 4, :],
                                     start=True, stop=False)
                    nc.tensor.matmul(psc[:, q * 512:(q + 1) * 512], w3c[:, :],
                                     B2C[:, 4 * q:4 * q + 4, :],
                                     start=False, stop=True)
                # DVE may read only one PSUM operand: evacuate the q1 half so
                # the C3 first-level max pairs psum against SBUF.
                c3h = kp.tile([128, 4, 128], F16, tag=f"c3h{rt % 2}")
                nc.scalar.activation(c3h[:, :, :],
                                     psc[:, 512:1024].rearrange("p (a b) -> p a b", a=4),
                                     COPY)
                BLK[rt] = (AC, B2C, psc, c3h)

            def emit_trees(rt):
                # k-max trees (fp16, 2x DVE), deferred one tile so every
                # input (relu evacs, conv3 psum) is long done when the DVE
                # stream reaches them — no cross-engine ping-pong stalls.
                sc, bl = rt // 4, rt % 4
                AC, B2C, psc, c3h = BLK.pop(rt)
                for (src, row0, lv) in ((AC, 0, 3), (B2C, 2 * G, 3), (psc, 3 * G, 2)):
                    if bl == 0:
                        rednew = kp.tile([128, NBL, 128], F16,
                                         tag=f"red{row0}{sc % 2}")
                        RED[(sc, row0)] = rednew
                    red = RED[(sc, row0)]
                    if lv == 3:
                        m1 = kp.tile([128, 4, 128], F16, tag=f"m1{row0}")
                        nc.vector.tensor_tensor(out=m1[:, :, :], in0=src[:, 0:4, :],
                                                in1=src[:, 4:8, :], op=MAX)
                    else:
                        m1 = kp.tile([128, 4, 128], F16, tag=f"m1{row0}")
                        nc.vector.tensor_tensor(
                            out=m1[:, :, :],
                            in0=src[:, 0:512].rearrange("p (a b) -> p a b", a=4),
                            in1=c3h[:, :, :],
                            op=MAX)
                    m2 = kp.tile([128, 2, 128], F16, tag=f"m2{row0}")
                    nc.vector.tensor_tensor(out=m2[:, :, :], in0=m1[:, 0:2, :],
                                            in1=m1[:, 2:4, :], op=MAX)
                    nc.vector.tensor_tensor(out=red[:, bl, :], in0=m2[:, 0, :],
                                            in1=m2[:, 1, :], op=MAX)

            def emit_om(sc):
                # cross-half merge (4x stt) + S bias + output DMAs, per sc.
                for (row0, add_s) in ((0, False), (2 * G, False), (3 * G, True)):
                    red = RED[(sc, row0)]
                    hi = kp.tile([64, NBL * 128], F16, tag=f"hi{row0}")
                    nc.scalar.activation(hi[:, :],
                                         red[64:128, :, :].rearrange("p a n -> p (a n)"),
                                         COPY)
                    om = kp.tile([64, NBL * 128], F16, tag=f"om{row0}")
                    nc.vector.tensor_tensor(
                        out=om[:, :],
                        in0=red[0:64, :, :].rearrange("p a n -> p (a n)"),
                        in1=hi[:, :], op=MAX)
                    if add_s:
                        om2 = kp.tile([64, NBL * 128], F16, tag="oms")
                        nc.vector.tensor_tensor(out=om2[:, :], in0=om[:, :],
                                                in1=SCt[:, sc * 512:(sc + 1) * 512],
                                                op=ADD)
                        om = om2
                    nc.sync.dma_start(out=Y[row0 if row0 else 0:(row0 if row0 else 0) + 64,
                                            sc * 512:(sc + 1) * 512],
                                      in_=om[:, :])

            # Interleaved emission: engines execute their streams in emission
            # order, so super-chunk work is placed one tile-group behind the
            # topk tiles whose indices it needs — sc_i's gather round-trip
            # hides behind tile group i+1's topk, and the table setup spreads
            # across groups 0 (P table) and 1 (T/R/S tables).
            for rt in range(NT):
                emit_topk(rt)
                # P table over tiles 0-2 (PT_D written before the first
                # gather's idx DMAs in queue order); T/R/S chunk u at tile 4u
                # (chunk u is first needed by block 4u at iteration 4u+3).
                if rt == 0:
                    emit_ptab(0)
                elif rt == 1:
                    emit_ptab(1)
                    emit_ptab(2)
                elif rt == 2:
                    emit_ptab(3)
                # pair gathers; pr=0 is deferred one tile so the PT_D table
                # write (ptab(3), tile 2) precedes it in DMA-queue order.
                if rt == 2:
                    emit_gather(0)
                elif rt % 2 == 1 and 3 <= rt <= 13:
                    emit_gather(rt // 2)
                elif rt >= 14:
                    emit_gather_single(rt)
                if rt >= 6:
                    emit_block(rt - 6)
                if rt >= 7:
                    emit_trees(rt - 7)
                    if (rt - 7) % 4 == 3:
                        emit_om((rt - 7) // 4)
                # T/R/S after block work: the PE reaches the DVE-blocking
                # transposes before burning time on the tables.
                if rt % 4 == 0:
                    emit_trs(rt // 4)
            for br in (NT - 6, NT - 5, NT - 4, NT - 3, NT - 2, NT - 1):
                emit_block(br)
                emit_trees(br - 1)
                if (br - 1) % 4 == 3:
                    emit_om((br - 1) // 4)
            emit_trees(NT - 1)
            emit_om(3)

    _fix_int_imms(nc)
    _split_all_waits(nc)
    _insert_gpsimd_library_load(nc, 3)
    return nc


def _prep_weights(W1, b1, W2, b2, W3, b3):
    W1 = np.asarray(W1, np.float32); W2 = np.asarray(W2, np.float32)
    W3 = np.asarray(W3, np.float32)
    b1 = np.asarray(b1, np.float32); b2 = np.asarray(b2, np.float32)
    b3 = np.asarray(b3, np.float32)
    W1a, W1b = W1[:, :64], W1[:, 64:]
    W2a, W2b = W2[:, :64], W2[:, 64:]
    W3a, W3b, W3c = W3[:, :64], W3[:, 64:128], W3[:, 128:]

    def blk(w):
        z = np.zeros((128, 128), np.float16)
        z[0:64, 0:64] = w.T
        z[64:128, 64:128] = w.T
        return z

    f16 = np.float16
    return {
        "WLTP": np.ascontiguousarray(W1a.T).astype(f16),
        "WLT": np.ascontiguousarray(np.vstack([(W1b - W1a).T, b1[None, :]])).astype(f16),
        "WLTD": np.ascontiguousarray(np.tile(np.vstack([(W1b - W1a).T, b1[None, :]]), (1, 2))).astype(f16),
        "WLR": np.ascontiguousarray(np.vstack([W2b.T, b2[None, :]])).astype(f16),
        "WLS": np.ascontiguousarray(np.vstack([W3b.T, b3[None, :]])).astype(f16),
        "W2BLK": blk(W2a),
        "W3ABLK": blk(W3a),
        "W3CBLK": blk(W3c),
        "EYE16": np.eye(128, dtype=f16),
        "EYE32": np.eye(128, dtype=np.float32),
        "IOTAI": np.tile(np.arange(N, dtype=np.int32), (128, 1)),
    }


_NC = None


def kernel(x, W1, b1, W2, b2, W3, b3):
    global _NC
    if _NC is None:
        _NC = build()
    x = np.asarray(x, np.float32)
    w = _prep_weights(W1, b1, W2, b2, W3, b3)
    in_maps = [{"x": np.ascontiguousarray(x[b]), **w} for b in range(B)]
    res = run_bass_kernel_spmd(_NC, in_maps, core_ids=list(range(B)))
    return np.stack([res.results[b]["y"].astype(np.float32) for b in range(B)], axis=0)
